# revision 54
# baseline (speedup 1.0000x reference)
"""Trainium2 Bass kernel for MatchingLayerL2:
   out = log_softmax(-sqrt(||x_i - y_j||^2) / std_j, axis=1)

x: [4096, 128] f32, y: [32768, 128] f32, std: [32768] f32 -> out [4096, 32768] f32.

Strategy: shard rows of x across 8 cores (512 rows each); y/std replicated.
Per core:
  rstd2_j = 1/std_j^2
  q_ij = rstd2_j * dist2_ij = (-2 x_i) . (y_j rstd2_j) + a_i rstd2_j + (b_j rstd2_j)
       (a = ||x||^2 rowwise, b = ||y hat||^2 * std^2 rowwise)
  s_ij = sqrt(q_ij) = dist_ij * rstd_j          (fp16 in SBUF)
  out_ij = -s_ij - ln(sum_j exp(-s_ij))          (no max-shift: s in [7,47])
Main matmul in bf16 (K=128); the rank-2 correction a*r + b*r is added with a
K=5 bf16 matmul whose rows are hi/lo bf16 splits for fp32-grade accuracy.
The 5 correction rows are staged through an internal DRAM tensor because a
[5, M] SBUF tile would charge M*2 bytes across all 128 partitions.

Schedule notes (tuned against the TimelineSim cost model; every feature
device-verified -- tensor_tensor_reduce and Pool tensor_scalar crash/wedge
the NeuronCore runtime and must not be used):
 - ACT (scalar) is the bottleneck: sqrt-from-PSUM and exp-with-accum are both
   mandatory full sweeps (~0.83 ns/elem each).  Everything else is kept under
   it: output written fp16 (halves DMA + runs the final axpy at DVE 4x),
   b-hat via one fused affine_mul_reduce per 128-column, half the yT staging
   copies on otherwise-idle ACT (Copy lives in every table set), -ln(S)
   entirely on DVE (bitcast exponent seed + cubic minimax for
   ln(m)-ln2*(m-1), |err|<=1e-3) so no table load or Newton exp sits
   between a block's last exp and the next block's Sqrt table load.
 - Engine SEQs are in-order and DMA instructions hold their queue while
   waiting, so program order ~= queue order.  Block 0's sqrt/exp (plus block
   1's first s-tile) and the corr loads are interleaved with the y-prologue
   super-chunks; rcorr chunk 0 is emitted before the first y super so its rt
   copies sit at the head of the DVE queue and ct(0) isn't starved.
 - PE matmul speed is decided at SEQ *visit* time from the length of PE's
   current busy-run (>3us full, >100ns mid, else low).  A SEQ blocked on a
   long wait re-visits with PE freshly restarted -> 788ns matmuls.  The
   phase-1 PSUM pools (3 mm bufs + transpose tiles) are closed after the
   prologue and steady state gets a 4-deep mm pool: the 4-tile runway of
   pre-computed matmuls absorbs the post-exp p-state ramp at each block
   transition.
 - SBUF is the binding constraint (~207.9 KB/partition).  Phase-1 staging
   pools live in a nested stack created last and closed after the prologue;
   the freed space hosts a 6th s-tile slot (sx_p, taken by block 1's st1)
   which decouples each block's first fresh s tile from the previous
   block's DMA-throttled out axpys (the out stream takes ~5.8us/s-tile).
 - The ~24us tail (last block's 8.4MB fp16 output after its logsumexp
   barrier) is DMA-bandwidth floor; the softmax normalizer makes it
   irreducible without approximating lnS.
"""

import os
import sys

sys.path.insert(0, "/root/.axon_site/_ro/trn_rl_repo")

import numpy as np
from contextlib import ExitStack

import concourse.bass as bass
from concourse import bacc
import concourse.tile as tile
from concourse.tile import add_dep_helper
from concourse import mybir, masks
from concourse.bass_utils import run_bass_kernel_spmd

F32 = mybir.dt.float32
BF16 = mybir.dt.bfloat16
FP16 = mybir.dt.float16
AF = mybir.ActivationFunctionType
ALU = mybir.AluOpType
AX = mybir.AxisListType

N_CORES = 8
D = 128
P = 128
# scheduling knobs (tuned against TimelineSim)
YT_MODE = 1   # yt copies: 0/1 alternate (ACT on that parity), 2 all DVE, 3 all ACT
YLOAD_SPLIT = 3   # split each y super-chunk load into (v+1) pieces
MM_BUFS = 3   # psum matmul tiles (2 banks each)
YSTAGE_BUFS = 2
OSTAGE_BUFS = 5
YBAR_BUFS = 2
ROWT_BUFS = 2
B1_EXP_AT7 = True
EARLY_JG = False
RC_ON_POOL = True
STDX_ON_POOL = False
NEWTON_ITERS = 2
PRO_EXPS = True   # emit phase-1 exps inside the prologue (False = batch after)
YT_FORCE_DVE = ()  # yt copies forced to DVE (tested: parity as-is wins)
CORRT_BUFS = 3
SX_AT = (1, 1)  # which steady s tile takes the extra 6th slot
JG0_FIRST = False  # tested: reordering jg0 before bhat g1 is worse
Y_ACT_SUPERS = 0  # tested: y loads via ACT hwdge queue are worse
STEADY_WIDE = False  # tested: 2-deep 2048 runway loses to 4-deep 1024
PIN_SQRT_TABLE = True  # dummy sqrt pins the initial act table to the Sqrt set
YB_INPLACE = False  # tested: in-place yb scale couples staging, loses 3.2us
YSTAGE_IP_BUFS = 3
PENDCT_K = 99  # prologue k at which to pre-emit block 1's ct(8); 99 = after loop
EXP_KS = (7, 15, 23, 31)  # prologue supers at which b0's s-tile exps run
ES_FP8 = True  # exp scratch output is discarded; fp8 halves its SBUF charge
BHAT_ONE_STORE = False  # tested: combined bhat store holds tp longer, +3.4us
TP_BUFS = 2   # psum transpose tiles (1 bank each); 2*MM_BUFS + TP_BUFS <= 8
MM_BUFS2 = 4  # steady-state psum matmul tiles (phase-1 psum pools are closed
              # first, freeing their banks); 2*MM_BUFS2 <= 8
NEGC_DVE = True  # -ln(S) fully on DVE (bit trick + cubic); keeps the Exp->Sqrt
                 # table load off the block-transition critical path
ST_ON_DVE = False  # DVE is not an hwdge engine on TRN2; stores stay on Pool
# c(m) = ln(m) - ln2*(m-1) on [1,2): minimax cubic, |err| <= 9.3e-4
LNC_A = (-0.78590174, 1.39373203, -0.71359, 0.10668473)


def build_nc(rows, M):
    """Build the Bass module for one core: x shard [rows, D], y [M, D], std [M]."""
    NB = rows // P          # row blocks of 128
    NSUP = M // 1024        # y super-chunks (1024 rows each) == 1024-col groups
    NST = M // 8192         # s tiles per block (8192 cols each)
    nA = M // P             # layout-A columns: v[q, t] = v[t*128 + q]

    nc = bacc.Bacc("TRN2", target_bir_lowering=False, debug=False, num_swdge_queues=4)
    x_d = nc.declare_dram_parameter("x", [rows, D], F32, isOutput=False)
    y_d = nc.declare_dram_parameter("y", [M, D], F32, isOutput=False)
    std_d = nc.declare_dram_parameter("std", [M], F32, isOutput=False)
    out_d = nc.declare_dram_parameter("out", [rows, M], FP16, isOutput=True)
    corr_d = nc.dram_tensor("corr", [5, M], BF16, kind="Internal")

    act_prev = [None]

    def act(*a, **k):
        inst = nc.scalar.activation(*a, **k)
        if act_prev[0] is not None:
            add_dep_helper(inst.ins, act_prev[0].ins, sync=False, reason="act order")
        act_prev[0] = inst
        return inst

    with tile.TileContext(nc) as tc, ExitStack() as ctx:
        pool = lambda name, bufs, space="SBUF": ctx.enter_context(
            tc.tile_pool(name=name, bufs=bufs, space=space)
        )

        # Persistent pools first; phase-1 staging pools go in their own stack
        # (created last = top of the SBUF stack) so closing them after the y
        # prologue frees real address space for the extra steady-state s slot.
        const_p = pool("const", 1)
        yT_p = pool("yT", NSUP)         # 32 x [128, 1024] bf16
        lhs_p = pool("lhs", 1)
        lhsc_p = pool("lhsc", NB)
        corrt_p = pool("corrt", CORRT_BUFS)
        s_p = pool("s", NST + 1)        # 5 x [128, 8192] fp16
        part_p = pool("part", 2)
        scal_p = pool("scal", 6)
        escr_p = pool("escr", 1)
        ostage_p = pool("ostage", OSTAGE_BUFS)  # [128, 1024] fp16

        stage1 = ctx.enter_context(ExitStack())
        spool = lambda name, bufs: stage1.enter_context(
            tc.tile_pool(name=name, bufs=bufs, space="SBUF")
        )
        ystage_p = spool("ystage", YSTAGE_IP_BUFS if YB_INPLACE else YSTAGE_BUFS)
        if not YB_INPLACE:
            ybar_p = spool("ybar", YBAR_BUFS)
        sq2_p = spool("sq2", 2)         # TTR product scratch [128, 128] bf16
        colsA_p = spool("colsA", 1)     # stdA, rstdA, rA, std2A  (f32 [128, nA])
        colsAh_p = spool("colsAh", 1)   # r hi/lo bf16 [128, nA]
        bcols_p = spool("bcols", 1)     # b2A f32 [128, nA]
        bg_p = spool("bg", 2)           # per-group bhat tiles [128, 16]
        rowT_p = spool("rowT", ROWT_BUFS)       # transposed row chunks [*, 128] bf16
        xa_p = spool("xa", 1)
        acol_p = spool("acol", 1)

        # phase-1 PSUM pools live in their own stack: closed after the y
        # prologue so the steady-state pool can take all 8 banks (deeper PE
        # runway over the block transition hides the p-state ramp).
        psum1 = ctx.enter_context(ExitStack())
        mm_ps = psum1.enter_context(
            tc.tile_pool(name="mmps", bufs=MM_BUFS, space="PSUM")
        )  # [128,1024] f32 = 2 banks each
        tp_ps = psum1.enter_context(
            tc.tile_pool(name="tpps", bufs=TP_BUFS, space="PSUM")
        )  # [128,1024] bf16 = 1 bank each
        MM = {"ps": mm_ps}

        # ---------------- constants ----------------
        ident = const_p.tile([P, P], BF16)
        masks.make_identity(nc, ident[:])
        identf = const_p.tile([P, P], F32)
        masks.make_identity(nc, identf[:])
        if PIN_SQRT_TABLE:
            # the first real ACT op is a Copy (in every table set); without a
            # hint the initial table load picks a non-Sqrt set and a second
            # 1283ns load lands on the startup critical path before the
            # first sqrt.  A 1-element dummy Sqrt pins the right set.
            tpin = scal_p.tile([P, 1], F32, tag="h")
            act(tpin[:], identf[:, 0:1], AF.Sqrt)

        # ---------------- std-derived quantities (layout A) ----------------
        # Minimal chain to rA first: the y prologue (ybar scaling) needs it.
        # stdA[q, t] = std[128 t + q]: load natural [t, q] tiles, PE-transpose.
        stdA = colsA_p.tile([P, nA], F32)
        for c in range((nA + P - 1) // P):
            h = min(P, nA - c * P)
            stn = rowT_p.tile([P, P], F32, tag="stn")
            (nc.gpsimd if STDX_ON_POOL else nc.sync).dma_start(
                out=stn[0:h, :],
                in_=std_d[P * P * c : P * (P * c + h)].rearrange(
                    "(t q) -> t q", q=P
                ),
            )
            tpf = tp_ps.tile([P, P], F32, tag="tp")
            nc.tensor.transpose(tpf[:, 0:h], stn[0:h, :], identf[:])
            nc.vector.tensor_copy(stdA[:, c * P : c * P + h], tpf[:, 0:h])
        # separate tiles: the in-place std->1/std->1/std^2 chain stalls the
        # DVE ~2.2us at startup on back-to-back same-region write-acks
        std2A = colsA_p.tile([P, nA], F32)
        nc.vector.tensor_tensor(std2A[:], stdA[:], stdA[:], op=ALU.mult)
        rstdA = colsA_p.tile([P, nA], F32)
        nc.vector.reciprocal(rstdA[:], stdA[:])
        rA = colsA_p.tile([P, nA], F32)
        nc.vector.tensor_tensor(rA[:], rstdA[:], rstdA[:], op=ALU.mult)
        rhiA = colsAh_p.tile([P, nA], BF16)
        nc.vector.tensor_copy(rhiA[:], rA[:])
        rloA = colsAh_p.tile([P, nA], BF16)
        nc.vector.tensor_tensor(rloA[:], rA[:], rhiA[:], op=ALU.subtract)

        # corr rows 0,1 = r_hi (pairs with a_hi, a_lo), row 2 = r_lo (pairs a_hi).
        # Transposed to row-major before storing.  Chunk c covers j columns
        # [16384 c, 16384 (c+1)); chunk 1 is deferred into phase 1 (only
        # needed from jg 15 on).  Stores ride the SP queue so the Pool queue
        # stays clear for phase-1 corr loads.
        def emit_rcorr_chunk(c):
            w = min(P, nA - c * P)
            for row, src in ((0, rhiA), (1, rhiA), (2, rloA)):
                tp = tp_ps.tile([P, 1024], BF16, tag="tp")
                nc.tensor.transpose(
                    tp[0:w, 0:P], src[:, c * P : c * P + w], ident[:]
                )
                rt = rowT_p.tile([P, P], BF16, tag="rowT")
                nc.vector.tensor_copy(rt[0:w, :], tp[0:w, 0:P])
                eng = (
                    nc.vector
                    if ST_ON_DVE
                    else (nc.gpsimd if RC_ON_POOL else nc.sync)
                )
                eng.dma_start(
                    out=corr_d[row, c * P * P : (c * P + w) * P].rearrange(
                        "(t q) -> t q", q=P
                    ),
                    in_=rt[0:w, :],
                )

        # ---------------- x side (emitted at phase-1 k==2) ----------------
        # lhsT_main = (-2x)^T bf16, a = ||x||^2.  Deferred into the super-chunk
        # loop so the first y super-chunks' DVE work isn't queued behind it
        # (engine queues are in-order); it's only needed by the first matmul.
        lhsT_main = lhs_p.tile([P, rows], BF16)
        lhsT_corr = []

        def emit_x_side():
            xstage = xa_p.tile([P, NB, D], F32)
            (nc.gpsimd if STDX_ON_POOL else nc.sync).dma_start(
                out=xstage[:], in_=x_d[:, :].rearrange("(c p) d -> p c d", p=P)
            )
            a_cols = acol_p.tile([P, NB], F32)
            for c in range(NB):
                xs2 = sq2_p.tile([P, D], BF16, tag="xs2")
                nc.vector.affine_mul_reduce(
                    out=xs2[:],
                    accum_out=a_cols[:, c : c + 1],
                    in0=xstage[:, c, :],
                    in1=xstage[:, c, :],
                    scale=1.0,
                    bias=0.0,
                )
            ahi_col = acol_p.tile([P, NB], BF16)
            nc.vector.tensor_copy(ahi_col[:], a_cols[:])
            alo_col = acol_p.tile([P, NB], BF16)
            nc.vector.tensor_tensor(alo_col[:], a_cols[:], ahi_col[:], op=ALU.subtract)

            xbar = xa_p.tile([P, NB, D], BF16, tag="xbar")
            nc.vector.tensor_scalar(xbar[:], xstage[:], -2.0, None, op0=ALU.mult)
            for c in range(NB):
                tp = tp_ps.tile([P, 1024], BF16, tag="tp")
                nc.tensor.transpose(tp[:, 0:P], xbar[:, c, :], ident[:])
                nc.vector.tensor_copy(lhsT_main[:, c * P : (c + 1) * P], tp[:, 0:P])

            # lhsT_corr per block: rows [a_hi; a_lo; a_hi; 1; 1] as [5, 128] bf16
            for b in range(NB):
                asm = acol_p.tile([P, 8], BF16, tag="asm")
                nc.vector.tensor_copy(asm[:, 0:1], ahi_col[:, b : b + 1])
                nc.vector.tensor_copy(asm[:, 1:2], alo_col[:, b : b + 1])
                nc.vector.tensor_copy(asm[:, 2:3], ahi_col[:, b : b + 1])
                nc.vector.memset(asm[:, 3:5], 1.0)
                tp = tp_ps.tile([P, 1024], BF16, tag="tp")
                nc.tensor.transpose(tp[0:5, 0:P], asm[:, 0:5], ident[:])
                lc = lhsc_p.tile([5, P], BF16)
                nc.vector.tensor_copy(lc[:], tp[0:5, 0:P])
                lhsT_corr.append(lc)

        # ---------------- shared emitters ----------------
        yT = []                 # 32 x [128, 1024] bf16 (super-chunk k)
        b2A = bcols_p.tile([P, nA], F32)

        def emit_y_super(k):
            """Load+scale+transpose y rows [1024k, 1024(k+1)); fill b-hat cols.

            b-hat comes from one fused affine_mul_reduce per 128-column:
            out = (yb * std2) * yb, accum = std2 * sum(yb^2) = ||yhat||^2 std^2.
            The yT staging copies alternate ACT/DVE: ACT has idle in phase 1
            (delivery-paced) and Copy lives in every activation table set."""
            yst = ystage_p.tile([P, 8, D], F32)
            # first supers ride the otherwise-idle ACT hwdge queue so the SP
            # queue (std + x) doesn't serialize the startup's y deliveries
            yq = nc.scalar if k < Y_ACT_SUPERS else nc.sync
            if YLOAD_SPLIT:
                # split loads: the ybar chain starts on the first piece
                # while the rest are still in flight
                nsp = YLOAD_SPLIT + 1
                assert 8 % nsp == 0, "YLOAD_SPLIT+1 must divide 8"
                cw = 8 // nsp
                for hh in range(nsp):
                    yq.dma_start(
                        out=yst[:, cw * hh : cw * (hh + 1), :],
                        in_=y_d[1024 * k + 128 * cw * hh : 1024 * k + 128 * cw * (hh + 1), :
                                ].rearrange("(c p) d -> p c d", p=P),
                    )
            else:
                yq.dma_start(
                    out=yst[:],
                    in_=y_d[1024 * k : 1024 * (k + 1), :].rearrange(
                        "(c p) d -> p c d", p=P
                    ),
                )
            if YB_INPLACE:
                # bf16 scale output written over the f32 stage bytes it just
                # read (per chunk: write bytes [512c,512c+256) trail the read
                # of [512c,512c+512)) -- the ybar staging pool disappears and
                # its 4KB funds a deeper ystage
                yb = yst[:].bitcast(BF16)[:, :, 0:D]
            else:
                yb = ybar_p.tile([P, 8, D], BF16)
            for c in range(8):
                nc.vector.tensor_scalar(
                    yb[:, c, :],
                    yst[:, c, :],
                    rA[:, 8 * k + c : 8 * k + c + 1],
                    None,
                    op0=ALU.mult,
                )
            for c in range(8):
                sq2 = sq2_p.tile([P, D], BF16)
                nc.vector.affine_mul_reduce(
                    out=sq2[:],
                    accum_out=b2A[:, 8 * k + c : 8 * k + c + 1],
                    in0=yb[:, c, :],
                    in1=yb[:, c, :],
                    scale=std2A[:, 8 * k + c : 8 * k + c + 1],
                    bias=0.0,
                )
            tp = tp_ps.tile([P, 1024], BF16, tag="tp")
            for c in range(8):
                nc.tensor.transpose(
                    tp[:, c * P : (c + 1) * P], yb[:, c, :], ident[:]
                )
            yt = yT_p.tile([P, 1024], BF16)
            # YT_MODE: 0/1 = alternate (ACT on that parity), 2 = all DVE,
            # 3 = all ACT.  ACT copies stay unchained: Copy lives in every
            # activation table set, and chaining would lock ACT progress to
            # y-super delivery.
            on_act = (YT_MODE == 3) or (YT_MODE in (0, 1) and k % 2 == YT_MODE)
            if k in YT_FORCE_DVE:
                # supers consumed right after a prologue exp: an ACT-queued
                # copy would sit behind the 7us exp and stall the next mms
                on_act = False
            if on_act:
                nc.scalar.copy(yt[:], tp[:])
            else:
                nc.vector.tensor_copy(yt[:], tp[:])
            yT.append(yt)

        def emit_bhat_group(g):
            """b-hat hi/lo rows for layout-A cols [16g, 16(g+1)) -> corr_d."""
            csl = slice(16 * g, 16 * (g + 1))
            bhi = bg_p.tile([P, 16], BF16, tag="bhi")
            nc.vector.tensor_copy(bhi[:], b2A[:, csl])
            blo = bg_p.tile([P, 16], BF16, tag="blo")
            nc.vector.tensor_tensor(blo[:], b2A[:, csl], bhi[:], op=ALU.subtract)
            if BHAT_ONE_STORE:
                # both rows stacked in one [32,128] tile -> a single DMA
                # (rows 3 and 4 of corr_d via a 2D pattern): halves the Pool
                # gen count feeding every ct load
                tp2 = tp_ps.tile([P, 1024], BF16, tag="tp")
                nc.tensor.transpose(tp2[0:16, 0:P], bhi[:], ident[:])
                # PE transpose writes need base partition 0/32/64
                nc.tensor.transpose(tp2[32:48, 0:P], blo[:], ident[:])
                rt = rowT_p.tile([P, P], BF16, tag="rowT")
                nc.vector.tensor_copy(rt[0:16, :], tp2[0:16, 0:P])
                nc.vector.tensor_copy(rt[16:32, :], tp2[32:48, 0:P])
                nc.gpsimd.dma_start(
                    out=corr_d[3:5, 2048 * g : 2048 * (g + 1)].rearrange(
                        "r (t q) -> r t q", q=P
                    ),
                    in_=rt[0:32, :].rearrange("(r t) q -> r t q", r=2),
                )
            else:
                for row, src in ((3, bhi), (4, blo)):
                    tp2 = tp_ps.tile([P, 1024], BF16, tag="tp")
                    nc.tensor.transpose(tp2[0:16, 0:P], src[:], ident[:])
                    rt = rowT_p.tile([P, P], BF16, tag="rowT")
                    nc.vector.tensor_copy(rt[0:16, :], tp2[0:16, 0:P])
                    (nc.vector if ST_ON_DVE else nc.gpsimd).dma_start(
                        out=corr_d[row, 2048 * g : 2048 * (g + 1)].rearrange(
                            "(t q) -> t q", q=P
                        ),
                        in_=rt[0:16, :],
                    )

        def emit_ct_load(jg):
            """corr rows for cols [1024 jg, 1024 (jg+2)); jg even."""
            ct = corrt_p.tile([5, 2048], BF16)
            nc.gpsimd.dma_start(
                out=ct[:], in_=corr_d[:, 1024 * jg : 1024 * (jg + 2)]
            )
            return ct

        def emit_mm_pair(b, jg0, ct):
            """Main+corr matmuls for col groups jg0, jg0+1 of block b.
            The ISA caps a matmul's moving free dim at 512, so each 1024-col
            group is two matmuls; all four mains run back-to-back before the
            four corrs so the stationary tensor only switches once per pair."""
            mms = []
            for i in range(2):
                mm = MM["ps"].tile([P, 1024], F32, tag="mm")
                for q in range(2):
                    nc.tensor.matmul(
                        mm[:, 512 * q : 512 * (q + 1)],
                        lhsT_main[:, b * P : (b + 1) * P],
                        yT[jg0 + i][:, 512 * q : 512 * (q + 1)],
                        start=True,
                        stop=False,
                    )
                mms.append(mm)
            for i in range(2):
                for q in range(2):
                    nc.tensor.matmul(
                        mms[i][:, 512 * q : 512 * (q + 1)],
                        lhsT_corr[b][:],
                        ct[:, 1024 * i + 512 * q : 1024 * i + 512 * (q + 1)],
                        start=False,
                        stop=True,
                    )
            return mms

        def emit_mm_quad(b, jg0, ct):
            """Steady-state variant: one [128, 2048] psum tile covering col
            groups jg0, jg0+1 (exactly one ct tile), consumed by ONE
            2048-wide sqrt -- halves the per-sqrt ACT access-latency
            charge."""
            mm = MM["ps"].tile([P, 2048], F32, tag="mm")
            for i in range(4):
                nc.tensor.matmul(
                    mm[:, 512 * i : 512 * (i + 1)],
                    lhsT_main[:, b * P : (b + 1) * P],
                    yT[jg0 + i // 2][:, 512 * (i % 2) : 512 * (i % 2 + 1)],
                    start=True,
                    stop=False,
                )
            for i in range(4):
                nc.tensor.matmul(
                    mm[:, 512 * i : 512 * (i + 1)],
                    lhsT_corr[b][:],
                    ct[:, 512 * i : 512 * (i + 1)],
                    start=False,
                    stop=True,
                )
            return mm

        def emit_sqrt(s_t, h, mm):
            act(s_t[:, 1024 * h : 1024 * (h + 1)], mm[:], AF.Sqrt)

        def emit_sqrt2(s_t, h2, mm):
            act(s_t[:, 2048 * h2 : 2048 * (h2 + 1)], mm[:], AF.Sqrt)

        def emit_exp(s_t, partials, st):
            es = escr_p.tile([P, 8192], mybir.dt.float8e4 if ES_FP8 else BF16)
            act(
                es[:],
                s_t[:],
                AF.Exp,
                scale=-1.0,
                accum_out=partials[:, st : st + 1],
            )

        LN2 = 0.6931471805599453

        def emit_lnS(partials):
            # negc = -ln(S), entirely on DVE so no Ln/Exp table dependency:
            # S = m 2^E; bits/2^23 = (E+127) + (m-1);
            # ln S = ln2*(bits/2^23 - 127) + c(m), c = ln m - ln2 (m-1)
            # (cubic minimax for c, |err| <= 9.3e-4 -> ~5e-5 rel on output).
            S = scal_p.tile([P, 1], F32)
            nc.vector.tensor_reduce(S[:], partials[:], axis=AX.X, op=ALU.add)
            if not NEGC_DVE:
                bits = scal_p.tile([P, 1], F32)
                nc.vector.tensor_copy(bits[:], S[:].bitcast(mybir.dt.int32))
                y = scal_p.tile([P, 1], F32, tag="y")
                nc.vector.tensor_scalar(
                    y[:], bits[:], LN2 / (1 << 23), -(127.0 - 0.043) * LN2,
                    op0=ALU.mult, op1=ALU.add,
                )
                for _ in range(NEWTON_ITERS):
                    e = scal_p.tile([P, 1], F32, tag="nwe")
                    act(e[:], y[:], AF.Exp, scale=-1.0)
                    t = scal_p.tile([P, 1], F32, tag="nwt")
                    nc.vector.tensor_tensor(t[:], e[:], S[:], op=ALU.mult)
                    y2 = scal_p.tile([P, 1], F32, tag="y")
                    nc.vector.tensor_tensor(y2[:], y[:], t[:], op=ALU.add)
                    y = scal_p.tile([P, 1], F32, tag="y")
                    nc.vector.tensor_scalar(y[:], y2[:], 1.0, None, op0=ALU.subtract)
                negc = scal_p.tile([P, 1], F32)
                nc.vector.tensor_scalar(negc[:], y[:], -1.0, None, op0=ALU.mult)
                return negc
            a0, a1, a2, a3 = LNC_A
            bits = scal_p.tile([P, 1], F32)
            nc.vector.tensor_copy(bits[:], S[:].bitcast(mybir.dt.int32))
            nb = scal_p.tile([P, 1], F32, tag="nb")
            nc.vector.tensor_scalar(
                nb[:], bits[:], -LN2 / (1 << 23), 127.0 * LN2,
                op0=ALU.mult, op1=ALU.add,
            )
            mb = scal_p.tile([P, 1], mybir.dt.int32, tag="mb")
            nc.vector.tensor_scalar(
                mb[:], S[:].bitcast(mybir.dt.int32), 0x007FFFFF, 0x3F800000,
                op0=ALU.bitwise_and, op1=ALU.bitwise_or,
            )
            m = mb[:].bitcast(F32)
            t = scal_p.tile([P, 1], F32, tag="h")
            nc.vector.tensor_scalar(t[:], m, -a3, -a2, op0=ALU.mult, op1=ALU.add)
            t2 = scal_p.tile([P, 1], F32, tag="h")
            nc.vector.tensor_tensor(t2[:], t[:], m, op=ALU.mult)
            t3 = scal_p.tile([P, 1], F32, tag="h")
            nc.vector.tensor_scalar(t3[:], t2[:], -a1, None, op0=ALU.add)
            t4 = scal_p.tile([P, 1], F32, tag="h")
            nc.vector.tensor_tensor(t4[:], t3[:], m, op=ALU.mult)
            t5 = scal_p.tile([P, 1], F32, tag="h")
            nc.vector.tensor_scalar(t5[:], t4[:], -a0, None, op0=ALU.add)
            negc = scal_p.tile([P, 1], F32)
            nc.vector.tensor_tensor(negc[:], nb[:], t5[:], op=ALU.add)
            return negc

        def emit_out(b, s_tiles, negc):
            for st in range(NST):
                for h in range(8):
                    og = ostage_p.tile([P, 1024], FP16)
                    nc.vector.tensor_scalar(
                        og[:],
                        s_tiles[st][:, 1024 * h : 1024 * (h + 1)],
                        -1.0,
                        negc[:],
                        op0=ALU.mult,
                        op1=ALU.add,
                    )
                    j0 = 8192 * st + 1024 * h
                    nc.sync.dma_start(
                        out=out_d[b * P : (b + 1) * P, j0 : j0 + 1024],
                        in_=og[:],
                    )

        # ---------------- phase 1: y prologue + block 0 (and b1 st0) ----------------
        # The first sqrts wait for the corr roundtrip (bhat group 0 store ->
        # ct load), so jg 0..3 are emitted as one batch at k==3; block 1's
        # first s-tile rides the same ct/yT deliveries (k < 8) to fill ACT.
        s0_tiles = []
        pending = [None]
        partials0 = part_p.tile([P, NST], F32)
        partials1 = part_p.tile([P, NST], F32)
        b1_st0 = None
        s_t = None
        for k in range(NSUP):
            if k == 0:
                # before the first y super: the rcorr rt copies land at the
                # head of the DVE queue (ahead of the y scales) so the Pool
                # stores' wait-at-head is short and ct(0) isn't starved
                emit_rcorr_chunk(0)
            emit_y_super(k)
            if k == 10:
                emit_rcorr_chunk(1)
            if k % 2 == 1:
                if not (JG0_FIRST and not EARLY_JG and k == 3):
                    emit_bhat_group((k - 1) // 2)
                if k == (1 if EARLY_JG else 3):
                    # after bhat group 0's DVE work so the corr roundtrip
                    # (which gates the first sqrt) isn't queued behind the
                    # x-side DVE chain
                    emit_x_side()
                if EARLY_JG and k == 1:
                    jgr = range(0, 2)
                elif k < 3:
                    continue
                elif k == 3:
                    jgr = range(0, 4) if not EARLY_JG else range(2, 4)
                else:
                    jgr = range(k - 1, k + 1)
                for jg in jgr:
                    if jg % 2 == 1:
                        continue
                    if JG0_FIRST and not EARLY_JG and k == 3 and jg == 2:
                        # ct(0) only needs bhat group 0: the jg 0-1 batch is
                        # emitted before group 1's stores so ct(0) isn't
                        # queued on Pool behind stores waiting on supers 2-3
                        emit_bhat_group(1)
                    ct = emit_ct_load(jg)
                    if jg % 8 == 0:
                        s_t = s_p.tile([P, 8192], FP16, tag="s_t")
                        s0_tiles.append(s_t)
                        if jg == 0:
                            # allocated after s0's first tile: pool rotation
                            # then lands block 1's later tiles on slots that
                            # free early
                            b1_st0 = s_p.tile([P, 8192], FP16, tag="s_t")
                    mms = emit_mm_pair(0, jg, ct)
                    emit_sqrt(s_t, jg % 8, mms[0])
                    emit_sqrt(s_t, jg % 8 + 1, mms[1])
                    if k < 8:
                        # block 1's first s-tile rides the same deliveries
                        mms1 = emit_mm_pair(1, jg, ct)
                        emit_sqrt(b1_st0, jg % 8, mms1[0])
                        emit_sqrt(b1_st0, jg % 8 + 1, mms1[1])
                if k == PENDCT_K:
                    # block 1's first steady ct: emitted mid-prologue so it
                    # sits AHEAD of the late cts (26..30) on the Pool queue;
                    # emitted at the back it transfers only after the k=31
                    # exp and block 1's corr matmuls restart cold
                    pending[0] = emit_ct_load(8)
                if PRO_EXPS == 1 and k in EXP_KS:
                    st_i = EXP_KS.index(k)
                    emit_exp(s0_tiles[st_i], partials0, st_i)
                    if st_i == 0:
                        emit_exp(b1_st0, partials1, 0)
                elif PRO_EXPS == 2 and k in (15, 31):
                    # pairs: one Exp-table residency per two s tiles
                    base = (k - 15) // 16 * 2
                    emit_exp(s0_tiles[base], partials0, base)
                    emit_exp(s0_tiles[base + 1], partials0, base + 1)
                    if k == 31:
                        emit_exp(b1_st0, partials1, 0)
        # block 1 resumes at jg 8; pre-load its corr tile while ACT runs the
        # phase-1 exp tail so the first steady-state sqrt isn't DMA-gated
        pending_ct = pending[0] if pending[0] is not None else emit_ct_load(8)
        # phase-1 PSUM (3 mm bufs + transpose tiles) closes here; steady
        # state gets a 4-deep mm pool so PE can pre-run a whole extra jg
        # pair across each block transition.  The SBUF staging pools close
        # too, freeing room for a 6th s-tile slot (sx_p) that decouples each
        # block's first fresh s tile from the previous block's out axpys.
        psum1.close()
        stage1.close()
        MM["ps"] = ctx.enter_context(
            tc.tile_pool(
                name="mmps2",
                bufs=2 if STEADY_WIDE else MM_BUFS2,
                space="PSUM",
            )
        )
        sx_p = ctx.enter_context(tc.tile_pool(name="sx", bufs=1, space="SBUF"))
        if not PRO_EXPS:
            # batched phase-1 exps: one Exp table residency instead of three
            # Sqrt<->Exp round trips inside the prologue
            for st in range(NST):
                emit_exp(s0_tiles[st], partials0, st)
            emit_exp(b1_st0, partials1, 0)
        negc0 = emit_lnS(partials0)
        emit_out(0, s0_tiles, negc0)

        # ---------------- blocks 1..NB-1 ----------------
        for b in range(1, NB):
            if b == 1:
                partials = partials1
                s_tiles = [b1_st0]
                st_range = range(1, NST)
            else:
                partials = part_p.tile([P, NST], F32)
                s_tiles = []
                st_range = range(NST)
            first_jg = 8 * st_range.start
            for st in st_range:
                sp = sx_p if (b, st) == SX_AT else s_p
                s_t = sp.tile([P, 8192], FP16, tag="s_t", name="s_t")
                for h2 in range(4):
                    jg = 8 * st + 2 * h2
                    ct = pending_ct if jg == first_jg else emit_ct_load(jg)
                    if STEADY_WIDE:
                        mm = emit_mm_quad(b, jg, ct)
                        emit_sqrt2(s_t, h2, mm)
                    else:
                        mms = emit_mm_pair(b, jg, ct)
                        emit_sqrt(s_t, 2 * h2, mms[0])
                        emit_sqrt(s_t, 2 * h2 + 1, mms[1])
                s_tiles.append(s_t)
            if b < NB - 1:
                # pre-load the next block's first corr tile during this
                # block's exp phase
                pending_ct = emit_ct_load(0)
            for st in st_range:
                emit_exp(s_tiles[st], partials, st)
            negc = emit_lnS(partials)
            emit_out(b, s_tiles, negc)

    nc.finalize()
    return nc


_NC_CACHE = {}


def _get_nc(rows, M):
    key = (rows, M)
    if key not in _NC_CACHE:
        _NC_CACHE[key] = build_nc(rows, M)
    return _NC_CACHE[key]


def kernel(x: np.ndarray, y: np.ndarray, std: np.ndarray) -> np.ndarray:
    x = np.ascontiguousarray(x, dtype=np.float32)
    y = np.ascontiguousarray(y, dtype=np.float32)
    std = np.ascontiguousarray(std, dtype=np.float32)
    N, M = x.shape[0], y.shape[0]
    rows = N // N_CORES
    nc = _get_nc(rows, M)
    in_maps = [
        {"x": x[c * rows : (c + 1) * rows], "y": y, "std": std}
        for c in range(N_CORES)
    ]
    trace = bool(int(os.environ.get("KERNEL_TRACE", "0")))
    res = run_bass_kernel_spmd(
        nc, in_maps, core_ids=list(range(N_CORES)), trace=trace
    )
    global LAST_RESULT
    LAST_RESULT = res
    return np.concatenate(
        [res.results[c]["out"] for c in range(N_CORES)], axis=0
    ).astype(np.float32)


LAST_RESULT = None



# revision 55
# speedup vs baseline: 1.0037x; 1.0037x over previous
"""Trainium2 Bass kernel for MatchingLayerL2:
   out = log_softmax(-sqrt(||x_i - y_j||^2) / std_j, axis=1)

x: [4096, 128] f32, y: [32768, 128] f32, std: [32768] f32 -> out [4096, 32768] f32.

Strategy: shard rows of x across 8 cores (512 rows each); y/std replicated.
Per core:
  rstd2_j = 1/std_j^2
  q_ij = rstd2_j * dist2_ij = (-2 x_i) . (y_j rstd2_j) + a_i rstd2_j + (b_j rstd2_j)
       (a = ||x||^2 rowwise, b = ||y hat||^2 * std^2 rowwise)
  s_ij = sqrt(q_ij) = dist_ij * rstd_j          (fp16 in SBUF)
  out_ij = -s_ij - ln(sum_j exp(-s_ij))          (no max-shift: s in [7,47])
Main matmul in bf16 (K=128); the rank-2 correction a*r + b*r is added with a
K=5 bf16 matmul whose rows are hi/lo bf16 splits for fp32-grade accuracy.
The 5 correction rows are staged through an internal DRAM tensor because a
[5, M] SBUF tile would charge M*2 bytes across all 128 partitions.

Schedule notes (tuned against the TimelineSim cost model; every feature
device-verified -- tensor_tensor_reduce and Pool tensor_scalar crash/wedge
the NeuronCore runtime and must not be used):
 - ACT (scalar) is the bottleneck: sqrt-from-PSUM and exp-with-accum are both
   mandatory full sweeps (~0.83 ns/elem each).  Everything else is kept under
   it: output written fp16 (halves DMA + runs the final axpy at DVE 4x),
   b-hat via one fused affine_mul_reduce per 128-column, half the yT staging
   copies on otherwise-idle ACT (Copy lives in every table set), -ln(S)
   entirely on DVE (bitcast exponent seed + cubic minimax for
   ln(m)-ln2*(m-1), |err|<=1e-3) so no table load or Newton exp sits
   between a block's last exp and the next block's Sqrt table load.
 - Engine SEQs are in-order and DMA instructions hold their queue while
   waiting, so program order ~= queue order.  Block 0's sqrt/exp (plus block
   1's first s-tile) and the corr loads are interleaved with the y-prologue
   super-chunks; rcorr chunk 0 is emitted before the first y super so its rt
   copies sit at the head of the DVE queue and ct(0) isn't starved.
 - PE matmul speed is decided at SEQ *visit* time from the length of PE's
   current busy-run (>3us full, >100ns mid, else low).  A SEQ blocked on a
   long wait re-visits with PE freshly restarted -> 788ns matmuls.  The
   phase-1 PSUM pools (3 mm bufs + transpose tiles) are closed after the
   prologue and steady state gets a 4-deep mm pool: the 4-tile runway of
   pre-computed matmuls absorbs the post-exp p-state ramp at each block
   transition.
 - SBUF is the binding constraint (~207.9 KB/partition).  Phase-1 staging
   pools live in a nested stack created last and closed after the prologue;
   the freed space hosts a 6th s-tile slot (sx_p, taken by block 1's st1)
   which decouples each block's first fresh s tile from the previous
   block's DMA-throttled out axpys (the out stream takes ~5.8us/s-tile).
 - The ~24us tail (last block's 8.4MB fp16 output after its logsumexp
   barrier) is DMA-bandwidth floor; the softmax normalizer makes it
   irreducible without approximating lnS.
"""

import os
import sys

sys.path.insert(0, "/root/.axon_site/_ro/trn_rl_repo")

import numpy as np
from contextlib import ExitStack

import concourse.bass as bass
from concourse import bacc
import concourse.tile as tile
from concourse.tile import add_dep_helper
from concourse import mybir, masks
from concourse.bass_utils import run_bass_kernel_spmd

F32 = mybir.dt.float32
BF16 = mybir.dt.bfloat16
FP16 = mybir.dt.float16
AF = mybir.ActivationFunctionType
ALU = mybir.AluOpType
AX = mybir.AxisListType

N_CORES = 8
D = 128
P = 128
# scheduling knobs (tuned against TimelineSim)
YT_MODE = 1   # yt copies: 0/1 alternate (ACT on that parity), 2 all DVE, 3 all ACT
YLOAD_SPLIT = 3   # split each y super-chunk load into (v+1) pieces
MM_BUFS = 3   # psum matmul tiles (2 banks each)
YSTAGE_BUFS = 2
OSTAGE_BUFS = 5
YBAR_BUFS = 2
ROWT_BUFS = 2
B1_EXP_AT7 = True
EARLY_JG = False
RC_ON_POOL = True
STDX_ON_POOL = False
NEWTON_ITERS = 2
PRO_EXPS = True   # emit phase-1 exps inside the prologue (False = batch after)
YT_FORCE_DVE = ()  # yt copies forced to DVE (tested: parity as-is wins)
CORRT_BUFS = 3
SX_AT = (1, 1)  # which steady s tile takes the extra 6th slot
JG0_FIRST = False  # tested: reordering jg0 before bhat g1 is worse
Y_ACT_SUPERS = 0  # tested: y loads via ACT hwdge queue are worse
STEADY_WIDE = False  # tested: 2-deep 2048 runway loses to 4-deep 1024
PIN_SQRT_TABLE = True  # dummy sqrt pins the initial act table to the Sqrt set
YB_INPLACE = False  # tested: in-place yb scale couples staging, loses 3.2us
YSTAGE_IP_BUFS = 3
PENDCT_K = 99  # prologue k at which to pre-emit block 1's ct(8); 99 = after loop
EXP_KS = (7, 15, 23, 31)  # prologue supers at which b0's s-tile exps run
ES_FP8 = True  # exp scratch output is discarded; fp8 halves its SBUF charge
BHAT_ONE_STORE = False  # tested: combined bhat store holds tp longer, +3.4us
TP_BUFS = 2   # psum transpose tiles (1 bank each); 2*MM_BUFS + TP_BUFS <= 8
MM_BUFS2 = 4  # steady-state psum matmul tiles (phase-1 psum pools are closed
              # first, freeing their banks); 2*MM_BUFS2 <= 8
NEGC_DVE = True  # -ln(S) fully on DVE (bit trick + cubic); keeps the Exp->Sqrt
                 # table load off the block-transition critical path
ST_ON_DVE = False  # DVE is not an hwdge engine on TRN2; stores stay on Pool
# c(m) = ln(m) - ln2*(m-1) on [1,2): minimax cubic, |err| <= 9.3e-4
LNC_A = (-0.78590174, 1.39373203, -0.71359, 0.10668473)


def build_nc(rows, M):
    """Build the Bass module for one core: x shard [rows, D], y [M, D], std [M]."""
    NB = rows // P          # row blocks of 128
    NSUP = M // 1024        # y super-chunks (1024 rows each) == 1024-col groups
    NST = M // 8192         # s tiles per block (8192 cols each)
    nA = M // P             # layout-A columns: v[q, t] = v[t*128 + q]

    nc = bacc.Bacc("TRN2", target_bir_lowering=False, debug=False, num_swdge_queues=4)
    x_d = nc.declare_dram_parameter("x", [rows, D], F32, isOutput=False)
    y_d = nc.declare_dram_parameter("y", [M, D], F32, isOutput=False)
    std_d = nc.declare_dram_parameter("std", [M], F32, isOutput=False)
    out_d = nc.declare_dram_parameter("out", [rows, M], FP16, isOutput=True)
    corr_d = nc.dram_tensor("corr", [5, M], BF16, kind="Internal")

    act_prev = [None]

    def act(*a, **k):
        inst = nc.scalar.activation(*a, **k)
        if act_prev[0] is not None:
            add_dep_helper(inst.ins, act_prev[0].ins, sync=False, reason="act order")
        act_prev[0] = inst
        return inst

    with tile.TileContext(nc) as tc, ExitStack() as ctx:
        pool = lambda name, bufs, space="SBUF": ctx.enter_context(
            tc.tile_pool(name=name, bufs=bufs, space=space)
        )

        # Persistent pools first; phase-1 staging pools go in their own stack
        # (created last = top of the SBUF stack) so closing them after the y
        # prologue frees real address space for the extra steady-state s slot.
        const_p = pool("const", 1)
        yT_p = pool("yT", NSUP)         # 32 x [128, 1024] bf16
        lhs_p = pool("lhs", 1)
        lhsc_p = pool("lhsc", NB)
        corrt_p = pool("corrt", CORRT_BUFS)
        s_p = pool("s", NST + 1)        # 5 x [128, 8192] fp16
        part_p = pool("part", 2)
        scal_p = pool("scal", 6)
        escr_p = pool("escr", 1)
        ostage_p = pool("ostage", OSTAGE_BUFS)  # [128, 1024] fp16

        stage1 = ctx.enter_context(ExitStack())
        spool = lambda name, bufs: stage1.enter_context(
            tc.tile_pool(name=name, bufs=bufs, space="SBUF")
        )
        ystage_p = spool("ystage", YSTAGE_IP_BUFS if YB_INPLACE else YSTAGE_BUFS)
        if not YB_INPLACE:
            ybar_p = spool("ybar", YBAR_BUFS)
        sq2_p = spool("sq2", 2)         # TTR product scratch [128, 128] bf16
        colsA_p = spool("colsA", 1)     # stdA, rstdA, rA, std2A  (f32 [128, nA])
        bcols_p = spool("bcols", 1)     # b2A f32 [128, nA]
        bg_p = spool("bg", 2)           # per-group bhat tiles [128, 16]
        rowT_p = spool("rowT", ROWT_BUFS)       # transposed row chunks [*, 128] bf16
        xa_p = spool("xa", 1)
        acol_p = spool("acol", 1)

        # phase-1 PSUM pools live in their own stack: closed after the y
        # prologue so the steady-state pool can take all 8 banks (deeper PE
        # runway over the block transition hides the p-state ramp).
        psum1 = ctx.enter_context(ExitStack())
        mm_ps = psum1.enter_context(
            tc.tile_pool(name="mmps", bufs=MM_BUFS, space="PSUM")
        )  # [128,1024] f32 = 2 banks each
        tp_ps = psum1.enter_context(
            tc.tile_pool(name="tpps", bufs=TP_BUFS, space="PSUM")
        )  # [128,1024] bf16 = 1 bank each
        MM = {"ps": mm_ps}

        # ---------------- constants ----------------
        ident = const_p.tile([P, P], BF16)
        masks.make_identity(nc, ident[:])
        identf = const_p.tile([P, P], F32)
        masks.make_identity(nc, identf[:])
        if PIN_SQRT_TABLE:
            # the first real ACT op is a Copy (in every table set); without a
            # hint the initial table load picks a non-Sqrt set and a second
            # 1283ns load lands on the startup critical path before the
            # first sqrt.  A 1-element dummy Sqrt pins the right set.
            tpin = scal_p.tile([P, 1], F32, tag="h")
            act(tpin[:], identf[:, 0:1], AF.Sqrt)

        # ---------------- std-derived quantities (layout A) ----------------
        # Minimal chain to rA first: the y prologue (ybar scaling) needs it.
        # stdA[q, t] = std[128 t + q]: load natural [t, q] tiles, PE-transpose.
        stdA = colsA_p.tile([P, nA], F32)
        stn_tiles = []
        for c in range((nA + P - 1) // P):
            h = min(P, nA - c * P)
            stn = rowT_p.tile([P, P], F32, tag="stn")
            (nc.gpsimd if STDX_ON_POOL else nc.sync).dma_start(
                out=stn[0:h, :],
                in_=std_d[P * P * c : P * (P * c + h)].rearrange(
                    "(t q) -> t q", q=P
                ),
            )
            stn_tiles.append(stn)
            tpf = tp_ps.tile([P, P], F32, tag="tp")
            nc.tensor.transpose(tpf[:, 0:h], stn[0:h, :], identf[:])
            nc.vector.tensor_copy(stdA[:, c * P : c * P + h], tpf[:, 0:h])
        # separate tiles: the in-place std->1/std->1/std^2 chain stalls the
        # DVE ~2.2us at startup on back-to-back same-region write-acks
        std2A = colsA_p.tile([P, nA], F32)
        nc.vector.tensor_tensor(std2A[:], stdA[:], stdA[:], op=ALU.mult)
        rstdA = colsA_p.tile([P, nA], F32)
        nc.vector.reciprocal(rstdA[:], stdA[:])
        rA = colsA_p.tile([P, nA], F32)
        nc.vector.tensor_tensor(rA[:], rstdA[:], rstdA[:], op=ALU.mult)
        # corr rows 0,1 = r_hi (pairs with a_hi, a_lo), row 2 = r_lo (pairs
        # a_hi).  Computed DIRECTLY on the natural-layout stn tiles (their
        # [t, q] order IS corr_d's row-major j order), so the stores depend
        # only on the std load -- not on the stdA-transpose/rA chain that
        # backs up the DVE queue at startup.
        def emit_rcorr_chunk(c):
            w = min(P, nA - c * P)
            stn = stn_tiles[c]
            rn = rowT_p.tile([P, P], F32, tag="rn")
            nc.vector.reciprocal(rn[0:w, :], stn[0:w, :])
            rq = rowT_p.tile([P, P], F32, tag="rq")
            nc.vector.tensor_tensor(rq[0:w, :], rn[0:w, :], rn[0:w, :], op=ALU.mult)
            rhi_n = rowT_p.tile([P, P], BF16, tag="rowT")
            nc.vector.tensor_copy(rhi_n[0:w, :], rq[0:w, :])
            rlo_n = rowT_p.tile([P, P], BF16, tag="rowT")
            nc.vector.tensor_tensor(
                rlo_n[0:w, :], rq[0:w, :], rhi_n[0:w, :], op=ALU.subtract
            )
            eng = nc.gpsimd if RC_ON_POOL else nc.sync
            for row, srcn in ((0, rhi_n), (1, rhi_n), (2, rlo_n)):
                eng.dma_start(
                    out=corr_d[row, c * P * P : (c * P + w) * P].rearrange(
                        "(t q) -> t q", q=P
                    ),
                    in_=srcn[0:w, :],
                )

        # ---------------- x side (emitted at phase-1 k==2) ----------------
        # lhsT_main = (-2x)^T bf16, a = ||x||^2.  Deferred into the super-chunk
        # loop so the first y super-chunks' DVE work isn't queued behind it
        # (engine queues are in-order); it's only needed by the first matmul.
        lhsT_main = lhs_p.tile([P, rows], BF16)
        lhsT_corr = []

        def emit_x_side():
            xstage = xa_p.tile([P, NB, D], F32)
            (nc.gpsimd if STDX_ON_POOL else nc.sync).dma_start(
                out=xstage[:], in_=x_d[:, :].rearrange("(c p) d -> p c d", p=P)
            )
            a_cols = acol_p.tile([P, NB], F32)
            for c in range(NB):
                xs2 = sq2_p.tile([P, D], BF16, tag="xs2")
                nc.vector.affine_mul_reduce(
                    out=xs2[:],
                    accum_out=a_cols[:, c : c + 1],
                    in0=xstage[:, c, :],
                    in1=xstage[:, c, :],
                    scale=1.0,
                    bias=0.0,
                )
            ahi_col = acol_p.tile([P, NB], BF16)
            nc.vector.tensor_copy(ahi_col[:], a_cols[:])
            alo_col = acol_p.tile([P, NB], BF16)
            nc.vector.tensor_tensor(alo_col[:], a_cols[:], ahi_col[:], op=ALU.subtract)

            xbar = xa_p.tile([P, NB, D], BF16, tag="xbar")
            nc.vector.tensor_scalar(xbar[:], xstage[:], -2.0, None, op0=ALU.mult)
            for c in range(NB):
                tp = tp_ps.tile([P, 1024], BF16, tag="tp")
                nc.tensor.transpose(tp[:, 0:P], xbar[:, c, :], ident[:])
                nc.vector.tensor_copy(lhsT_main[:, c * P : (c + 1) * P], tp[:, 0:P])

            # lhsT_corr per block: rows [a_hi; a_lo; a_hi; 1; 1] as [5, 128] bf16
            for b in range(NB):
                asm = acol_p.tile([P, 8], BF16, tag="asm")
                nc.vector.tensor_copy(asm[:, 0:1], ahi_col[:, b : b + 1])
                nc.vector.tensor_copy(asm[:, 1:2], alo_col[:, b : b + 1])
                nc.vector.tensor_copy(asm[:, 2:3], ahi_col[:, b : b + 1])
                nc.vector.memset(asm[:, 3:5], 1.0)
                tp = tp_ps.tile([P, 1024], BF16, tag="tp")
                nc.tensor.transpose(tp[0:5, 0:P], asm[:, 0:5], ident[:])
                lc = lhsc_p.tile([5, P], BF16)
                nc.vector.tensor_copy(lc[:], tp[0:5, 0:P])
                lhsT_corr.append(lc)

        # ---------------- shared emitters ----------------
        yT = []                 # 32 x [128, 1024] bf16 (super-chunk k)
        b2A = bcols_p.tile([P, nA], F32)

        def emit_y_super(k):
            """Load+scale+transpose y rows [1024k, 1024(k+1)); fill b-hat cols.

            b-hat comes from one fused affine_mul_reduce per 128-column:
            out = (yb * std2) * yb, accum = std2 * sum(yb^2) = ||yhat||^2 std^2.
            The yT staging copies alternate ACT/DVE: ACT has idle in phase 1
            (delivery-paced) and Copy lives in every activation table set."""
            yst = ystage_p.tile([P, 8, D], F32)
            # first supers ride the otherwise-idle ACT hwdge queue so the SP
            # queue (std + x) doesn't serialize the startup's y deliveries
            yq = nc.scalar if k < Y_ACT_SUPERS else nc.sync
            if YLOAD_SPLIT:
                # split loads: the ybar chain starts on the first piece
                # while the rest are still in flight
                nsp = YLOAD_SPLIT + 1
                assert 8 % nsp == 0, "YLOAD_SPLIT+1 must divide 8"
                cw = 8 // nsp
                for hh in range(nsp):
                    yq.dma_start(
                        out=yst[:, cw * hh : cw * (hh + 1), :],
                        in_=y_d[1024 * k + 128 * cw * hh : 1024 * k + 128 * cw * (hh + 1), :
                                ].rearrange("(c p) d -> p c d", p=P),
                    )
            else:
                yq.dma_start(
                    out=yst[:],
                    in_=y_d[1024 * k : 1024 * (k + 1), :].rearrange(
                        "(c p) d -> p c d", p=P
                    ),
                )
            if YB_INPLACE:
                # bf16 scale output written over the f32 stage bytes it just
                # read (per chunk: write bytes [512c,512c+256) trail the read
                # of [512c,512c+512)) -- the ybar staging pool disappears and
                # its 4KB funds a deeper ystage
                yb = yst[:].bitcast(BF16)[:, :, 0:D]
            else:
                yb = ybar_p.tile([P, 8, D], BF16)
            for c in range(8):
                nc.vector.tensor_scalar(
                    yb[:, c, :],
                    yst[:, c, :],
                    rA[:, 8 * k + c : 8 * k + c + 1],
                    None,
                    op0=ALU.mult,
                )
            for c in range(8):
                sq2 = sq2_p.tile([P, D], BF16)
                nc.vector.affine_mul_reduce(
                    out=sq2[:],
                    accum_out=b2A[:, 8 * k + c : 8 * k + c + 1],
                    in0=yb[:, c, :],
                    in1=yb[:, c, :],
                    scale=std2A[:, 8 * k + c : 8 * k + c + 1],
                    bias=0.0,
                )
            tp = tp_ps.tile([P, 1024], BF16, tag="tp")
            for c in range(8):
                nc.tensor.transpose(
                    tp[:, c * P : (c + 1) * P], yb[:, c, :], ident[:]
                )
            yt = yT_p.tile([P, 1024], BF16)
            # YT_MODE: 0/1 = alternate (ACT on that parity), 2 = all DVE,
            # 3 = all ACT.  ACT copies stay unchained: Copy lives in every
            # activation table set, and chaining would lock ACT progress to
            # y-super delivery.
            on_act = (YT_MODE == 3) or (YT_MODE in (0, 1) and k % 2 == YT_MODE)
            if k in YT_FORCE_DVE:
                # supers consumed right after a prologue exp: an ACT-queued
                # copy would sit behind the 7us exp and stall the next mms
                on_act = False
            if on_act:
                nc.scalar.copy(yt[:], tp[:])
            else:
                nc.vector.tensor_copy(yt[:], tp[:])
            yT.append(yt)

        def emit_bhat_group(g):
            """b-hat hi/lo rows for layout-A cols [16g, 16(g+1)) -> corr_d."""
            csl = slice(16 * g, 16 * (g + 1))
            bhi = bg_p.tile([P, 16], BF16, tag="bhi")
            nc.vector.tensor_copy(bhi[:], b2A[:, csl])
            blo = bg_p.tile([P, 16], BF16, tag="blo")
            nc.vector.tensor_tensor(blo[:], b2A[:, csl], bhi[:], op=ALU.subtract)
            if BHAT_ONE_STORE:
                # both rows stacked in one [32,128] tile -> a single DMA
                # (rows 3 and 4 of corr_d via a 2D pattern): halves the Pool
                # gen count feeding every ct load
                tp2 = tp_ps.tile([P, 1024], BF16, tag="tp")
                nc.tensor.transpose(tp2[0:16, 0:P], bhi[:], ident[:])
                # PE transpose writes need base partition 0/32/64
                nc.tensor.transpose(tp2[32:48, 0:P], blo[:], ident[:])
                rt = rowT_p.tile([P, P], BF16, tag="rowT")
                nc.vector.tensor_copy(rt[0:16, :], tp2[0:16, 0:P])
                nc.vector.tensor_copy(rt[16:32, :], tp2[32:48, 0:P])
                nc.gpsimd.dma_start(
                    out=corr_d[3:5, 2048 * g : 2048 * (g + 1)].rearrange(
                        "r (t q) -> r t q", q=P
                    ),
                    in_=rt[0:32, :].rearrange("(r t) q -> r t q", r=2),
                )
            else:
                for row, src in ((3, bhi), (4, blo)):
                    tp2 = tp_ps.tile([P, 1024], BF16, tag="tp")
                    nc.tensor.transpose(tp2[0:16, 0:P], src[:], ident[:])
                    rt = rowT_p.tile([P, P], BF16, tag="rowT")
                    nc.vector.tensor_copy(rt[0:16, :], tp2[0:16, 0:P])
                    (nc.vector if ST_ON_DVE else nc.gpsimd).dma_start(
                        out=corr_d[row, 2048 * g : 2048 * (g + 1)].rearrange(
                            "(t q) -> t q", q=P
                        ),
                        in_=rt[0:16, :],
                    )

        def emit_ct_load(jg):
            """corr rows for cols [1024 jg, 1024 (jg+2)); jg even."""
            ct = corrt_p.tile([5, 2048], BF16)
            nc.gpsimd.dma_start(
                out=ct[:], in_=corr_d[:, 1024 * jg : 1024 * (jg + 2)]
            )
            return ct

        def emit_mm_pair(b, jg0, ct):
            """Main+corr matmuls for col groups jg0, jg0+1 of block b.
            The ISA caps a matmul's moving free dim at 512, so each 1024-col
            group is two matmuls; all four mains run back-to-back before the
            four corrs so the stationary tensor only switches once per pair."""
            mms = []
            for i in range(2):
                mm = MM["ps"].tile([P, 1024], F32, tag="mm")
                for q in range(2):
                    nc.tensor.matmul(
                        mm[:, 512 * q : 512 * (q + 1)],
                        lhsT_main[:, b * P : (b + 1) * P],
                        yT[jg0 + i][:, 512 * q : 512 * (q + 1)],
                        start=True,
                        stop=False,
                    )
                mms.append(mm)
            for i in range(2):
                for q in range(2):
                    nc.tensor.matmul(
                        mms[i][:, 512 * q : 512 * (q + 1)],
                        lhsT_corr[b][:],
                        ct[:, 1024 * i + 512 * q : 1024 * i + 512 * (q + 1)],
                        start=False,
                        stop=True,
                    )
            return mms

        def emit_mm_quad(b, jg0, ct):
            """Steady-state variant: one [128, 2048] psum tile covering col
            groups jg0, jg0+1 (exactly one ct tile), consumed by ONE
            2048-wide sqrt -- halves the per-sqrt ACT access-latency
            charge."""
            mm = MM["ps"].tile([P, 2048], F32, tag="mm")
            for i in range(4):
                nc.tensor.matmul(
                    mm[:, 512 * i : 512 * (i + 1)],
                    lhsT_main[:, b * P : (b + 1) * P],
                    yT[jg0 + i // 2][:, 512 * (i % 2) : 512 * (i % 2 + 1)],
                    start=True,
                    stop=False,
                )
            for i in range(4):
                nc.tensor.matmul(
                    mm[:, 512 * i : 512 * (i + 1)],
                    lhsT_corr[b][:],
                    ct[:, 512 * i : 512 * (i + 1)],
                    start=False,
                    stop=True,
                )
            return mm

        def emit_sqrt(s_t, h, mm):
            act(s_t[:, 1024 * h : 1024 * (h + 1)], mm[:], AF.Sqrt)

        def emit_sqrt2(s_t, h2, mm):
            act(s_t[:, 2048 * h2 : 2048 * (h2 + 1)], mm[:], AF.Sqrt)

        def emit_exp(s_t, partials, st):
            es = escr_p.tile([P, 8192], mybir.dt.float8e4 if ES_FP8 else BF16)
            act(
                es[:],
                s_t[:],
                AF.Exp,
                scale=-1.0,
                accum_out=partials[:, st : st + 1],
            )

        LN2 = 0.6931471805599453

        def emit_lnS(partials):
            # negc = -ln(S), entirely on DVE so no Ln/Exp table dependency:
            # S = m 2^E; bits/2^23 = (E+127) + (m-1);
            # ln S = ln2*(bits/2^23 - 127) + c(m), c = ln m - ln2 (m-1)
            # (cubic minimax for c, |err| <= 9.3e-4 -> ~5e-5 rel on output).
            S = scal_p.tile([P, 1], F32)
            nc.vector.tensor_reduce(S[:], partials[:], axis=AX.X, op=ALU.add)
            if not NEGC_DVE:
                bits = scal_p.tile([P, 1], F32)
                nc.vector.tensor_copy(bits[:], S[:].bitcast(mybir.dt.int32))
                y = scal_p.tile([P, 1], F32, tag="y")
                nc.vector.tensor_scalar(
                    y[:], bits[:], LN2 / (1 << 23), -(127.0 - 0.043) * LN2,
                    op0=ALU.mult, op1=ALU.add,
                )
                for _ in range(NEWTON_ITERS):
                    e = scal_p.tile([P, 1], F32, tag="nwe")
                    act(e[:], y[:], AF.Exp, scale=-1.0)
                    t = scal_p.tile([P, 1], F32, tag="nwt")
                    nc.vector.tensor_tensor(t[:], e[:], S[:], op=ALU.mult)
                    y2 = scal_p.tile([P, 1], F32, tag="y")
                    nc.vector.tensor_tensor(y2[:], y[:], t[:], op=ALU.add)
                    y = scal_p.tile([P, 1], F32, tag="y")
                    nc.vector.tensor_scalar(y[:], y2[:], 1.0, None, op0=ALU.subtract)
                negc = scal_p.tile([P, 1], F32)
                nc.vector.tensor_scalar(negc[:], y[:], -1.0, None, op0=ALU.mult)
                return negc
            a0, a1, a2, a3 = LNC_A
            bits = scal_p.tile([P, 1], F32)
            nc.vector.tensor_copy(bits[:], S[:].bitcast(mybir.dt.int32))
            nb = scal_p.tile([P, 1], F32, tag="nb")
            nc.vector.tensor_scalar(
                nb[:], bits[:], -LN2 / (1 << 23), 127.0 * LN2,
                op0=ALU.mult, op1=ALU.add,
            )
            mb = scal_p.tile([P, 1], mybir.dt.int32, tag="mb")
            nc.vector.tensor_scalar(
                mb[:], S[:].bitcast(mybir.dt.int32), 0x007FFFFF, 0x3F800000,
                op0=ALU.bitwise_and, op1=ALU.bitwise_or,
            )
            m = mb[:].bitcast(F32)
            t = scal_p.tile([P, 1], F32, tag="h")
            nc.vector.tensor_scalar(t[:], m, -a3, -a2, op0=ALU.mult, op1=ALU.add)
            t2 = scal_p.tile([P, 1], F32, tag="h")
            nc.vector.tensor_tensor(t2[:], t[:], m, op=ALU.mult)
            t3 = scal_p.tile([P, 1], F32, tag="h")
            nc.vector.tensor_scalar(t3[:], t2[:], -a1, None, op0=ALU.add)
            t4 = scal_p.tile([P, 1], F32, tag="h")
            nc.vector.tensor_tensor(t4[:], t3[:], m, op=ALU.mult)
            t5 = scal_p.tile([P, 1], F32, tag="h")
            nc.vector.tensor_scalar(t5[:], t4[:], -a0, None, op0=ALU.add)
            negc = scal_p.tile([P, 1], F32)
            nc.vector.tensor_tensor(negc[:], nb[:], t5[:], op=ALU.add)
            return negc

        def emit_out(b, s_tiles, negc):
            for st in range(NST):
                for h in range(8):
                    og = ostage_p.tile([P, 1024], FP16)
                    nc.vector.tensor_scalar(
                        og[:],
                        s_tiles[st][:, 1024 * h : 1024 * (h + 1)],
                        -1.0,
                        negc[:],
                        op0=ALU.mult,
                        op1=ALU.add,
                    )
                    j0 = 8192 * st + 1024 * h
                    nc.sync.dma_start(
                        out=out_d[b * P : (b + 1) * P, j0 : j0 + 1024],
                        in_=og[:],
                    )

        # ---------------- phase 1: y prologue + block 0 (and b1 st0) ----------------
        # The first sqrts wait for the corr roundtrip (bhat group 0 store ->
        # ct load), so jg 0..3 are emitted as one batch at k==3; block 1's
        # first s-tile rides the same ct/yT deliveries (k < 8) to fill ACT.
        s0_tiles = []
        pending = [None]
        partials0 = part_p.tile([P, NST], F32)
        partials1 = part_p.tile([P, NST], F32)
        b1_st0 = None
        s_t = None
        for k in range(NSUP):
            if k == 0:
                # before the first y super: the rcorr rt copies land at the
                # head of the DVE queue (ahead of the y scales) so the Pool
                # stores' wait-at-head is short and ct(0) isn't starved
                emit_rcorr_chunk(0)
            emit_y_super(k)
            if k == 10:
                emit_rcorr_chunk(1)
            if k % 2 == 1:
                if not (JG0_FIRST and not EARLY_JG and k == 3):
                    emit_bhat_group((k - 1) // 2)
                if k == (1 if EARLY_JG else 3):
                    # after bhat group 0's DVE work so the corr roundtrip
                    # (which gates the first sqrt) isn't queued behind the
                    # x-side DVE chain
                    emit_x_side()
                if EARLY_JG and k == 1:
                    jgr = range(0, 2)
                elif k < 3:
                    continue
                elif k == 3:
                    jgr = range(0, 4) if not EARLY_JG else range(2, 4)
                else:
                    jgr = range(k - 1, k + 1)
                for jg in jgr:
                    if jg % 2 == 1:
                        continue
                    if JG0_FIRST and not EARLY_JG and k == 3 and jg == 2:
                        # ct(0) only needs bhat group 0: the jg 0-1 batch is
                        # emitted before group 1's stores so ct(0) isn't
                        # queued on Pool behind stores waiting on supers 2-3
                        emit_bhat_group(1)
                    ct = emit_ct_load(jg)
                    if jg % 8 == 0:
                        s_t = s_p.tile([P, 8192], FP16, tag="s_t")
                        s0_tiles.append(s_t)
                        if jg == 0:
                            # allocated after s0's first tile: pool rotation
                            # then lands block 1's later tiles on slots that
                            # free early
                            b1_st0 = s_p.tile([P, 8192], FP16, tag="s_t")
                    mms = emit_mm_pair(0, jg, ct)
                    emit_sqrt(s_t, jg % 8, mms[0])
                    emit_sqrt(s_t, jg % 8 + 1, mms[1])
                    if k < 8:
                        # block 1's first s-tile rides the same deliveries
                        mms1 = emit_mm_pair(1, jg, ct)
                        emit_sqrt(b1_st0, jg % 8, mms1[0])
                        emit_sqrt(b1_st0, jg % 8 + 1, mms1[1])
                if k == PENDCT_K:
                    # block 1's first steady ct: emitted mid-prologue so it
                    # sits AHEAD of the late cts (26..30) on the Pool queue;
                    # emitted at the back it transfers only after the k=31
                    # exp and block 1's corr matmuls restart cold
                    pending[0] = emit_ct_load(8)
                if PRO_EXPS == 1 and k in EXP_KS:
                    st_i = EXP_KS.index(k)
                    emit_exp(s0_tiles[st_i], partials0, st_i)
                    if st_i == 0:
                        emit_exp(b1_st0, partials1, 0)
                elif PRO_EXPS == 2 and k in (15, 31):
                    # pairs: one Exp-table residency per two s tiles
                    base = (k - 15) // 16 * 2
                    emit_exp(s0_tiles[base], partials0, base)
                    emit_exp(s0_tiles[base + 1], partials0, base + 1)
                    if k == 31:
                        emit_exp(b1_st0, partials1, 0)
        # block 1 resumes at jg 8; pre-load its corr tile while ACT runs the
        # phase-1 exp tail so the first steady-state sqrt isn't DMA-gated
        pending_ct = pending[0] if pending[0] is not None else emit_ct_load(8)
        # phase-1 PSUM (3 mm bufs + transpose tiles) closes here; steady
        # state gets a 4-deep mm pool so PE can pre-run a whole extra jg
        # pair across each block transition.  The SBUF staging pools close
        # too, freeing room for a 6th s-tile slot (sx_p) that decouples each
        # block's first fresh s tile from the previous block's out axpys.
        psum1.close()
        stage1.close()
        MM["ps"] = ctx.enter_context(
            tc.tile_pool(
                name="mmps2",
                bufs=2 if STEADY_WIDE else MM_BUFS2,
                space="PSUM",
            )
        )
        sx_p = ctx.enter_context(tc.tile_pool(name="sx", bufs=1, space="SBUF"))
        if not PRO_EXPS:
            # batched phase-1 exps: one Exp table residency instead of three
            # Sqrt<->Exp round trips inside the prologue
            for st in range(NST):
                emit_exp(s0_tiles[st], partials0, st)
            emit_exp(b1_st0, partials1, 0)
        negc0 = emit_lnS(partials0)
        emit_out(0, s0_tiles, negc0)

        # ---------------- blocks 1..NB-1 ----------------
        for b in range(1, NB):
            if b == 1:
                partials = partials1
                s_tiles = [b1_st0]
                st_range = range(1, NST)
            else:
                partials = part_p.tile([P, NST], F32)
                s_tiles = []
                st_range = range(NST)
            first_jg = 8 * st_range.start
            for st in st_range:
                sp = sx_p if (b, st) == SX_AT else s_p
                s_t = sp.tile([P, 8192], FP16, tag="s_t", name="s_t")
                for h2 in range(4):
                    jg = 8 * st + 2 * h2
                    ct = pending_ct if jg == first_jg else emit_ct_load(jg)
                    if STEADY_WIDE:
                        mm = emit_mm_quad(b, jg, ct)
                        emit_sqrt2(s_t, h2, mm)
                    else:
                        mms = emit_mm_pair(b, jg, ct)
                        emit_sqrt(s_t, 2 * h2, mms[0])
                        emit_sqrt(s_t, 2 * h2 + 1, mms[1])
                s_tiles.append(s_t)
            if b < NB - 1:
                # pre-load the next block's first corr tile during this
                # block's exp phase
                pending_ct = emit_ct_load(0)
            for st in st_range:
                emit_exp(s_tiles[st], partials, st)
            negc = emit_lnS(partials)
            emit_out(b, s_tiles, negc)

    nc.finalize()
    return nc


_NC_CACHE = {}


def _get_nc(rows, M):
    key = (rows, M)
    if key not in _NC_CACHE:
        _NC_CACHE[key] = build_nc(rows, M)
    return _NC_CACHE[key]


def kernel(x: np.ndarray, y: np.ndarray, std: np.ndarray) -> np.ndarray:
    x = np.ascontiguousarray(x, dtype=np.float32)
    y = np.ascontiguousarray(y, dtype=np.float32)
    std = np.ascontiguousarray(std, dtype=np.float32)
    N, M = x.shape[0], y.shape[0]
    rows = N // N_CORES
    nc = _get_nc(rows, M)
    in_maps = [
        {"x": x[c * rows : (c + 1) * rows], "y": y, "std": std}
        for c in range(N_CORES)
    ]
    trace = bool(int(os.environ.get("KERNEL_TRACE", "0")))
    res = run_bass_kernel_spmd(
        nc, in_maps, core_ids=list(range(N_CORES)), trace=trace
    )
    global LAST_RESULT
    LAST_RESULT = res
    return np.concatenate(
        [res.results[c]["out"] for c in range(N_CORES)], axis=0
    ).astype(np.float32)


LAST_RESULT = None



# revision 58
# speedup vs baseline: 1.0038x; 1.0001x over previous
"""Trainium2 Bass kernel for MatchingLayerL2:
   out = log_softmax(-sqrt(||x_i - y_j||^2) / std_j, axis=1)

x: [4096, 128] f32, y: [32768, 128] f32, std: [32768] f32 -> out [4096, 32768] f32.

Strategy: shard rows of x across 8 cores (512 rows each); y/std replicated.
Per core:
  rstd2_j = 1/std_j^2
  q_ij = rstd2_j * dist2_ij = (-2 x_i) . (y_j rstd2_j) + a_i rstd2_j + (b_j rstd2_j)
       (a = ||x||^2 rowwise, b = ||y hat||^2 * std^2 rowwise)
  s_ij = sqrt(q_ij) = dist_ij * rstd_j          (fp16 in SBUF)
  out_ij = -s_ij - ln(sum_j exp(-s_ij))          (no max-shift: s in [7,47])
Main matmul in bf16 (K=128); the rank-2 correction a*r + b*r is added with a
K=5 bf16 matmul whose rows are hi/lo bf16 splits for fp32-grade accuracy.
The 5 correction rows are staged through an internal DRAM tensor because a
[5, M] SBUF tile would charge M*2 bytes across all 128 partitions.

Schedule notes (tuned against the TimelineSim cost model; every feature
device-verified -- tensor_tensor_reduce and Pool tensor_scalar crash/wedge
the NeuronCore runtime and must not be used):
 - ACT (scalar) is the bottleneck: sqrt-from-PSUM and exp-with-accum are both
   mandatory full sweeps (~0.83 ns/elem each).  Everything else is kept under
   it: output written fp16 (halves DMA + runs the final axpy at DVE 4x),
   b-hat via one fused affine_mul_reduce per 128-column, half the yT staging
   copies on otherwise-idle ACT (Copy lives in every table set), -ln(S)
   entirely on DVE (bitcast exponent seed + cubic minimax for
   ln(m)-ln2*(m-1), |err|<=1e-3) so no table load or Newton exp sits
   between a block's last exp and the next block's Sqrt table load.
 - Engine SEQs are in-order and DMA instructions hold their queue while
   waiting, so program order ~= queue order.  Block 0's sqrt/exp (plus block
   1's first s-tile) and the corr loads are interleaved with the y-prologue
   super-chunks; rcorr chunk 0 is emitted before the first y super so its rt
   copies sit at the head of the DVE queue and ct(0) isn't starved.
 - PE matmul speed is decided at SEQ *visit* time from the length of PE's
   current busy-run (>3us full, >100ns mid, else low).  A SEQ blocked on a
   long wait re-visits with PE freshly restarted -> 788ns matmuls.  The
   phase-1 PSUM pools (3 mm bufs + transpose tiles) are closed after the
   prologue and steady state gets a 4-deep mm pool: the 4-tile runway of
   pre-computed matmuls absorbs the post-exp p-state ramp at each block
   transition.
 - SBUF is the binding constraint (~207.9 KB/partition).  Phase-1 staging
   pools live in a nested stack created last and closed after the prologue;
   the freed space hosts a 6th s-tile slot (sx_p, taken by block 1's st1)
   which decouples each block's first fresh s tile from the previous
   block's DMA-throttled out axpys (the out stream takes ~5.8us/s-tile).
 - The ~24us tail (last block's 8.4MB fp16 output after its logsumexp
   barrier) is DMA-bandwidth floor; the softmax normalizer makes it
   irreducible without approximating lnS.
"""

import os
import sys

sys.path.insert(0, "/root/.axon_site/_ro/trn_rl_repo")

import numpy as np
from contextlib import ExitStack

import concourse.bass as bass
from concourse import bacc
import concourse.tile as tile
from concourse.tile import add_dep_helper
from concourse import mybir, masks
from concourse.bass_utils import run_bass_kernel_spmd

F32 = mybir.dt.float32
BF16 = mybir.dt.bfloat16
FP16 = mybir.dt.float16
AF = mybir.ActivationFunctionType
ALU = mybir.AluOpType
AX = mybir.AxisListType

N_CORES = 8
D = 128
P = 128
# scheduling knobs (tuned against TimelineSim)
YT_MODE = 1   # yt copies: 0/1 alternate (ACT on that parity), 2 all DVE, 3 all ACT
YLOAD_SPLIT = 3   # split each y super-chunk load into (v+1) pieces
MM_BUFS = 3   # psum matmul tiles (2 banks each)
YSTAGE_BUFS = 2
OSTAGE_BUFS = 5
YBAR_BUFS = 2
ROWT_BUFS = 2
B1_EXP_AT7 = True
EARLY_JG = False
RC_ON_POOL = True
STDX_ON_POOL = False
NEWTON_ITERS = 2
PRO_EXPS = True   # emit phase-1 exps inside the prologue (False = batch after)
YT_FORCE_DVE = ()  # yt copies forced to DVE (tested: parity as-is wins)
CORRT_BUFS = 3
SX_AT = (1, 1)  # which steady s tile takes the extra 6th slot
JG0_FIRST = False  # tested: reordering jg0 before bhat g1 is worse
Y_ACT_SUPERS = 0  # tested: y loads via ACT hwdge queue are worse
STEADY_WIDE = False  # tested: 2-deep 2048 runway loses to 4-deep 1024
PIN_SQRT_TABLE = True  # dummy sqrt pins the initial act table to the Sqrt set
YB_INPLACE = False  # tested: in-place yb scale couples staging, loses 3.2us
YSTAGE_IP_BUFS = 3
PENDCT_K = 99  # prologue k at which to pre-emit block 1's ct(8); 99 = after loop
EXP_KS = (7, 15, 23, 31)  # prologue supers at which b0's s-tile exps run
ES_FP8 = True  # exp scratch output is discarded; fp8 halves its SBUF charge
BHAT_ONE_STORE = False  # tested: combined bhat store holds tp longer, +3.4us
STD1_K = -1  # std chunk 1 upfront (deferred emission tested worse)
TP_BUFS = 2   # psum transpose tiles (1 bank each); 2*MM_BUFS + TP_BUFS <= 8
MM_BUFS2 = 4  # steady-state psum matmul tiles (phase-1 psum pools are closed
              # first, freeing their banks); 2*MM_BUFS2 <= 8
NEGC_DVE = True  # -ln(S) fully on DVE (bit trick + cubic); keeps the Exp->Sqrt
                 # table load off the block-transition critical path
ST_ON_DVE = False  # DVE is not an hwdge engine on TRN2; stores stay on Pool
# c(m) = ln(m) - ln2*(m-1) on [1,2): minimax cubic, |err| <= 9.3e-4
LNC_A = (-0.78590174, 1.39373203, -0.71359, 0.10668473)


def build_nc(rows, M):
    """Build the Bass module for one core: x shard [rows, D], y [M, D], std [M]."""
    NB = rows // P          # row blocks of 128
    NSUP = M // 1024        # y super-chunks (1024 rows each) == 1024-col groups
    NST = M // 8192         # s tiles per block (8192 cols each)
    nA = M // P             # layout-A columns: v[q, t] = v[t*128 + q]

    nc = bacc.Bacc("TRN2", target_bir_lowering=False, debug=False, num_swdge_queues=4)
    x_d = nc.declare_dram_parameter("x", [rows, D], F32, isOutput=False)
    y_d = nc.declare_dram_parameter("y", [M, D], F32, isOutput=False)
    std_d = nc.declare_dram_parameter("std", [M], F32, isOutput=False)
    out_d = nc.declare_dram_parameter("out", [rows, M], FP16, isOutput=True)
    corr_d = nc.dram_tensor("corr", [5, M], BF16, kind="Internal")

    act_prev = [None]

    def act(*a, **k):
        inst = nc.scalar.activation(*a, **k)
        if act_prev[0] is not None:
            add_dep_helper(inst.ins, act_prev[0].ins, sync=False, reason="act order")
        act_prev[0] = inst
        return inst

    with tile.TileContext(nc) as tc, ExitStack() as ctx:
        pool = lambda name, bufs, space="SBUF": ctx.enter_context(
            tc.tile_pool(name=name, bufs=bufs, space=space)
        )

        # Persistent pools first; phase-1 staging pools go in their own stack
        # (created last = top of the SBUF stack) so closing them after the y
        # prologue frees real address space for the extra steady-state s slot.
        const_p = pool("const", 1)
        yT_p = pool("yT", NSUP)         # 32 x [128, 1024] bf16
        lhs_p = pool("lhs", 1)
        lhsc_p = pool("lhsc", NB)
        corrt_p = pool("corrt", CORRT_BUFS)
        s_p = pool("s", NST + 1)        # 5 x [128, 8192] fp16
        part_p = pool("part", 2)
        scal_p = pool("scal", 6)
        escr_p = pool("escr", 1)
        ostage_p = pool("ostage", OSTAGE_BUFS)  # [128, 1024] fp16

        stage1 = ctx.enter_context(ExitStack())
        spool = lambda name, bufs: stage1.enter_context(
            tc.tile_pool(name=name, bufs=bufs, space="SBUF")
        )
        ystage_p = spool("ystage", YSTAGE_IP_BUFS if YB_INPLACE else YSTAGE_BUFS)
        if not YB_INPLACE:
            ybar_p = spool("ybar", YBAR_BUFS)
        sq2_p = spool("sq2", 2)         # TTR product scratch [128, 128] bf16
        colsA_p = spool("colsA", 1)     # stdA, rstdA, rA, std2A  (f32 [128, nA])
        bcols_p = spool("bcols", 1)     # b2A f32 [128, nA]
        bg_p = spool("bg", 2)           # per-group bhat tiles [128, 16]
        rowT_p = spool("rowT", ROWT_BUFS)       # transposed row chunks [*, 128] bf16
        xa_p = spool("xa", 1)
        acol_p = spool("acol", 1)

        # phase-1 PSUM pools live in their own stack: closed after the y
        # prologue so the steady-state pool can take all 8 banks (deeper PE
        # runway over the block transition hides the p-state ramp).
        psum1 = ctx.enter_context(ExitStack())
        mm_ps = psum1.enter_context(
            tc.tile_pool(name="mmps", bufs=MM_BUFS, space="PSUM")
        )  # [128,1024] f32 = 2 banks each
        tp_ps = psum1.enter_context(
            tc.tile_pool(name="tpps", bufs=TP_BUFS, space="PSUM")
        )  # [128,1024] bf16 = 1 bank each
        MM = {"ps": mm_ps}

        # ---------------- constants ----------------
        ident = const_p.tile([P, P], BF16)
        masks.make_identity(nc, ident[:])
        identf = const_p.tile([P, P], F32)
        masks.make_identity(nc, identf[:])
        if PIN_SQRT_TABLE:
            # the first real ACT op is a Copy (in every table set); without a
            # hint the initial table load picks a non-Sqrt set and a second
            # 1283ns load lands on the startup critical path before the
            # first sqrt.  A 1-element dummy Sqrt pins the right set.
            tpin = scal_p.tile([P, 1], F32, tag="h")
            act(tpin[:], identf[:, 0:1], AF.Sqrt)

        # ---------------- std-derived quantities (layout A) ----------------
        # Minimal chain to rA first: the y prologue (ybar scaling) needs it.
        # stdA[q, t] = std[128 t + q]: load natural [t, q] tiles, PE-transpose.
        stdA = colsA_p.tile([P, nA], F32)
        std2A = colsA_p.tile([P, nA], F32)
        rstdA = colsA_p.tile([P, nA], F32)
        rA = colsA_p.tile([P, nA], F32)
        stn_tiles = []

        def emit_std_chunk(c):
            # per-chunk std->rA chain: chunk 1 is deferred past the first y
            # supers (its rA cols feed supers >= 16; its stn feeds rcorr
            # chunk 1 at k==10) so it doesn't serialize ahead of y0/y1 on SP
            h = min(P, nA - c * P)
            stn = rowT_p.tile([P, P], F32, tag="stn")
            (nc.gpsimd if STDX_ON_POOL else nc.sync).dma_start(
                out=stn[0:h, :],
                in_=std_d[P * P * c : P * (P * c + h)].rearrange(
                    "(t q) -> t q", q=P
                ),
            )
            stn_tiles.append(stn)
            tpf = tp_ps.tile([P, P], F32, tag="tp")
            nc.tensor.transpose(tpf[:, 0:h], stn[0:h, :], identf[:])
            csl = slice(c * P, c * P + h)
            nc.vector.tensor_copy(stdA[:, csl], tpf[:, 0:h])
            nc.vector.tensor_tensor(
                std2A[:, csl], stdA[:, csl], stdA[:, csl], op=ALU.mult
            )
            nc.vector.reciprocal(rstdA[:, csl], stdA[:, csl])
            nc.vector.tensor_tensor(
                rA[:, csl], rstdA[:, csl], rstdA[:, csl], op=ALU.mult
            )

        emit_std_chunk(0)
        if STD1_K < 0:
            emit_std_chunk(1)
        # corr rows 0,1 = r_hi (pairs with a_hi, a_lo), row 2 = r_lo (pairs
        # a_hi).  Computed DIRECTLY on the natural-layout stn tiles (their
        # [t, q] order IS corr_d's row-major j order), so the stores depend
        # only on the std load -- not on the stdA-transpose/rA chain that
        # backs up the DVE queue at startup.
        def emit_rcorr_chunk(c):
            w = min(P, nA - c * P)
            stn = stn_tiles[c]
            rn = rowT_p.tile([P, P], F32, tag="rn")
            nc.vector.reciprocal(rn[0:w, :], stn[0:w, :])
            rq = rowT_p.tile([P, P], F32, tag="rq")
            nc.vector.tensor_tensor(rq[0:w, :], rn[0:w, :], rn[0:w, :], op=ALU.mult)
            rhi_n = rowT_p.tile([P, P], BF16, tag="rowT")
            nc.vector.tensor_copy(rhi_n[0:w, :], rq[0:w, :])
            rlo_n = rowT_p.tile([P, P], BF16, tag="rowT")
            nc.vector.tensor_tensor(
                rlo_n[0:w, :], rq[0:w, :], rhi_n[0:w, :], op=ALU.subtract
            )
            eng = nc.gpsimd if RC_ON_POOL else nc.sync
            for row, srcn in ((0, rhi_n), (1, rhi_n), (2, rlo_n)):
                eng.dma_start(
                    out=corr_d[row, c * P * P : (c * P + w) * P].rearrange(
                        "(t q) -> t q", q=P
                    ),
                    in_=srcn[0:w, :],
                )

        # ---------------- x side (emitted at phase-1 k==2) ----------------
        # lhsT_main = (-2x)^T bf16, a = ||x||^2.  Deferred into the super-chunk
        # loop so the first y super-chunks' DVE work isn't queued behind it
        # (engine queues are in-order); it's only needed by the first matmul.
        lhsT_main = lhs_p.tile([P, rows], BF16)
        lhsT_corr = []

        def emit_x_side():
            xstage = xa_p.tile([P, NB, D], F32)
            (nc.gpsimd if STDX_ON_POOL else nc.sync).dma_start(
                out=xstage[:], in_=x_d[:, :].rearrange("(c p) d -> p c d", p=P)
            )
            a_cols = acol_p.tile([P, NB], F32)
            for c in range(NB):
                xs2 = sq2_p.tile([P, D], BF16, tag="xs2")
                nc.vector.affine_mul_reduce(
                    out=xs2[:],
                    accum_out=a_cols[:, c : c + 1],
                    in0=xstage[:, c, :],
                    in1=xstage[:, c, :],
                    scale=1.0,
                    bias=0.0,
                )
            ahi_col = acol_p.tile([P, NB], BF16)
            nc.vector.tensor_copy(ahi_col[:], a_cols[:])
            alo_col = acol_p.tile([P, NB], BF16)
            nc.vector.tensor_tensor(alo_col[:], a_cols[:], ahi_col[:], op=ALU.subtract)

            xbar = xa_p.tile([P, NB, D], BF16, tag="xbar")
            nc.vector.tensor_scalar(xbar[:], xstage[:], -2.0, None, op0=ALU.mult)
            for c in range(NB):
                tp = tp_ps.tile([P, 1024], BF16, tag="tp")
                nc.tensor.transpose(tp[:, 0:P], xbar[:, c, :], ident[:])
                nc.vector.tensor_copy(lhsT_main[:, c * P : (c + 1) * P], tp[:, 0:P])

            # lhsT_corr per block: rows [a_hi; a_lo; a_hi; 1; 1] as [5, 128] bf16
            for b in range(NB):
                asm = acol_p.tile([P, 8], BF16, tag="asm")
                nc.vector.tensor_copy(asm[:, 0:1], ahi_col[:, b : b + 1])
                nc.vector.tensor_copy(asm[:, 1:2], alo_col[:, b : b + 1])
                nc.vector.tensor_copy(asm[:, 2:3], ahi_col[:, b : b + 1])
                nc.vector.memset(asm[:, 3:5], 1.0)
                tp = tp_ps.tile([P, 1024], BF16, tag="tp")
                nc.tensor.transpose(tp[0:5, 0:P], asm[:, 0:5], ident[:])
                lc = lhsc_p.tile([5, P], BF16)
                nc.vector.tensor_copy(lc[:], tp[0:5, 0:P])
                lhsT_corr.append(lc)

        # ---------------- shared emitters ----------------
        yT = []                 # 32 x [128, 1024] bf16 (super-chunk k)
        b2A = bcols_p.tile([P, nA], F32)

        def emit_y_super(k):
            """Load+scale+transpose y rows [1024k, 1024(k+1)); fill b-hat cols.

            b-hat comes from one fused affine_mul_reduce per 128-column:
            out = (yb * std2) * yb, accum = std2 * sum(yb^2) = ||yhat||^2 std^2.
            The yT staging copies alternate ACT/DVE: ACT has idle in phase 1
            (delivery-paced) and Copy lives in every activation table set."""
            yst = ystage_p.tile([P, 8, D], F32)
            # first supers ride the otherwise-idle ACT hwdge queue so the SP
            # queue (std + x) doesn't serialize the startup's y deliveries
            yq = nc.scalar if k < Y_ACT_SUPERS else nc.sync
            if YLOAD_SPLIT:
                # split loads: the ybar chain starts on the first piece
                # while the rest are still in flight
                nsp = YLOAD_SPLIT + 1
                assert 8 % nsp == 0, "YLOAD_SPLIT+1 must divide 8"
                cw = 8 // nsp
                for hh in range(nsp):
                    yq.dma_start(
                        out=yst[:, cw * hh : cw * (hh + 1), :],
                        in_=y_d[1024 * k + 128 * cw * hh : 1024 * k + 128 * cw * (hh + 1), :
                                ].rearrange("(c p) d -> p c d", p=P),
                    )
            else:
                yq.dma_start(
                    out=yst[:],
                    in_=y_d[1024 * k : 1024 * (k + 1), :].rearrange(
                        "(c p) d -> p c d", p=P
                    ),
                )
            if YB_INPLACE:
                # bf16 scale output written over the f32 stage bytes it just
                # read (per chunk: write bytes [512c,512c+256) trail the read
                # of [512c,512c+512)) -- the ybar staging pool disappears and
                # its 4KB funds a deeper ystage
                yb = yst[:].bitcast(BF16)[:, :, 0:D]
            else:
                yb = ybar_p.tile([P, 8, D], BF16)
            for c in range(8):
                nc.vector.tensor_scalar(
                    yb[:, c, :],
                    yst[:, c, :],
                    rA[:, 8 * k + c : 8 * k + c + 1],
                    None,
                    op0=ALU.mult,
                )
            for c in range(8):
                sq2 = sq2_p.tile([P, D], BF16)
                nc.vector.affine_mul_reduce(
                    out=sq2[:],
                    accum_out=b2A[:, 8 * k + c : 8 * k + c + 1],
                    in0=yb[:, c, :],
                    in1=yb[:, c, :],
                    scale=std2A[:, 8 * k + c : 8 * k + c + 1],
                    bias=0.0,
                )
            tp = tp_ps.tile([P, 1024], BF16, tag="tp")
            for c in range(8):
                nc.tensor.transpose(
                    tp[:, c * P : (c + 1) * P], yb[:, c, :], ident[:]
                )
            yt = yT_p.tile([P, 1024], BF16)
            # YT_MODE: 0/1 = alternate (ACT on that parity), 2 = all DVE,
            # 3 = all ACT.  ACT copies stay unchained: Copy lives in every
            # activation table set, and chaining would lock ACT progress to
            # y-super delivery.
            on_act = (YT_MODE == 3) or (YT_MODE in (0, 1) and k % 2 == YT_MODE)
            if k in YT_FORCE_DVE:
                # supers consumed right after a prologue exp: an ACT-queued
                # copy would sit behind the 7us exp and stall the next mms
                on_act = False
            if on_act:
                nc.scalar.copy(yt[:], tp[:])
            else:
                nc.vector.tensor_copy(yt[:], tp[:])
            yT.append(yt)

        def emit_bhat_group(g):
            """b-hat hi/lo rows for layout-A cols [16g, 16(g+1)) -> corr_d."""
            csl = slice(16 * g, 16 * (g + 1))
            bhi = bg_p.tile([P, 16], BF16, tag="bhi")
            nc.vector.tensor_copy(bhi[:], b2A[:, csl])
            blo = bg_p.tile([P, 16], BF16, tag="blo")
            nc.vector.tensor_tensor(blo[:], b2A[:, csl], bhi[:], op=ALU.subtract)
            if BHAT_ONE_STORE:
                # both rows stacked in one [32,128] tile -> a single DMA
                # (rows 3 and 4 of corr_d via a 2D pattern): halves the Pool
                # gen count feeding every ct load
                tp2 = tp_ps.tile([P, 1024], BF16, tag="tp")
                nc.tensor.transpose(tp2[0:16, 0:P], bhi[:], ident[:])
                # PE transpose writes need base partition 0/32/64
                nc.tensor.transpose(tp2[32:48, 0:P], blo[:], ident[:])
                rt = rowT_p.tile([P, P], BF16, tag="rowT")
                nc.vector.tensor_copy(rt[0:16, :], tp2[0:16, 0:P])
                nc.vector.tensor_copy(rt[16:32, :], tp2[32:48, 0:P])
                nc.gpsimd.dma_start(
                    out=corr_d[3:5, 2048 * g : 2048 * (g + 1)].rearrange(
                        "r (t q) -> r t q", q=P
                    ),
                    in_=rt[0:32, :].rearrange("(r t) q -> r t q", r=2),
                )
            else:
                for row, src in ((3, bhi), (4, blo)):
                    tp2 = tp_ps.tile([P, 1024], BF16, tag="tp")
                    nc.tensor.transpose(tp2[0:16, 0:P], src[:], ident[:])
                    rt = rowT_p.tile([P, P], BF16, tag="rowT")
                    nc.vector.tensor_copy(rt[0:16, :], tp2[0:16, 0:P])
                    (nc.vector if ST_ON_DVE else nc.gpsimd).dma_start(
                        out=corr_d[row, 2048 * g : 2048 * (g + 1)].rearrange(
                            "(t q) -> t q", q=P
                        ),
                        in_=rt[0:16, :],
                    )

        def emit_ct_load(jg):
            """corr rows for cols [1024 jg, 1024 (jg+2)); jg even."""
            ct = corrt_p.tile([5, 2048], BF16)
            nc.gpsimd.dma_start(
                out=ct[:], in_=corr_d[:, 1024 * jg : 1024 * (jg + 2)]
            )
            return ct

        def emit_mm_pair(b, jg0, ct):
            """Main+corr matmuls for col groups jg0, jg0+1 of block b.
            The ISA caps a matmul's moving free dim at 512, so each 1024-col
            group is two matmuls; all four mains run back-to-back before the
            four corrs so the stationary tensor only switches once per pair."""
            mms = []
            for i in range(2):
                mm = MM["ps"].tile([P, 1024], F32, tag="mm")
                for q in range(2):
                    nc.tensor.matmul(
                        mm[:, 512 * q : 512 * (q + 1)],
                        lhsT_main[:, b * P : (b + 1) * P],
                        yT[jg0 + i][:, 512 * q : 512 * (q + 1)],
                        start=True,
                        stop=False,
                    )
                mms.append(mm)
            for i in range(2):
                for q in range(2):
                    nc.tensor.matmul(
                        mms[i][:, 512 * q : 512 * (q + 1)],
                        lhsT_corr[b][:],
                        ct[:, 1024 * i + 512 * q : 1024 * i + 512 * (q + 1)],
                        start=False,
                        stop=True,
                    )
            return mms

        def emit_mm_quad(b, jg0, ct):
            """Steady-state variant: one [128, 2048] psum tile covering col
            groups jg0, jg0+1 (exactly one ct tile), consumed by ONE
            2048-wide sqrt -- halves the per-sqrt ACT access-latency
            charge."""
            mm = MM["ps"].tile([P, 2048], F32, tag="mm")
            for i in range(4):
                nc.tensor.matmul(
                    mm[:, 512 * i : 512 * (i + 1)],
                    lhsT_main[:, b * P : (b + 1) * P],
                    yT[jg0 + i // 2][:, 512 * (i % 2) : 512 * (i % 2 + 1)],
                    start=True,
                    stop=False,
                )
            for i in range(4):
                nc.tensor.matmul(
                    mm[:, 512 * i : 512 * (i + 1)],
                    lhsT_corr[b][:],
                    ct[:, 512 * i : 512 * (i + 1)],
                    start=False,
                    stop=True,
                )
            return mm

        def emit_sqrt(s_t, h, mm):
            act(s_t[:, 1024 * h : 1024 * (h + 1)], mm[:], AF.Sqrt)

        def emit_sqrt2(s_t, h2, mm):
            act(s_t[:, 2048 * h2 : 2048 * (h2 + 1)], mm[:], AF.Sqrt)

        def emit_exp(s_t, partials, st):
            es = escr_p.tile([P, 8192], mybir.dt.float8e4 if ES_FP8 else BF16)
            act(
                es[:],
                s_t[:],
                AF.Exp,
                scale=-1.0,
                accum_out=partials[:, st : st + 1],
            )

        LN2 = 0.6931471805599453

        def emit_lnS(partials):
            # negc = -ln(S), entirely on DVE so no Ln/Exp table dependency:
            # S = m 2^E; bits/2^23 = (E+127) + (m-1);
            # ln S = ln2*(bits/2^23 - 127) + c(m), c = ln m - ln2 (m-1)
            # (cubic minimax for c, |err| <= 9.3e-4 -> ~5e-5 rel on output).
            S = scal_p.tile([P, 1], F32)
            nc.vector.tensor_reduce(S[:], partials[:], axis=AX.X, op=ALU.add)
            if not NEGC_DVE:
                bits = scal_p.tile([P, 1], F32)
                nc.vector.tensor_copy(bits[:], S[:].bitcast(mybir.dt.int32))
                y = scal_p.tile([P, 1], F32, tag="y")
                nc.vector.tensor_scalar(
                    y[:], bits[:], LN2 / (1 << 23), -(127.0 - 0.043) * LN2,
                    op0=ALU.mult, op1=ALU.add,
                )
                for _ in range(NEWTON_ITERS):
                    e = scal_p.tile([P, 1], F32, tag="nwe")
                    act(e[:], y[:], AF.Exp, scale=-1.0)
                    t = scal_p.tile([P, 1], F32, tag="nwt")
                    nc.vector.tensor_tensor(t[:], e[:], S[:], op=ALU.mult)
                    y2 = scal_p.tile([P, 1], F32, tag="y")
                    nc.vector.tensor_tensor(y2[:], y[:], t[:], op=ALU.add)
                    y = scal_p.tile([P, 1], F32, tag="y")
                    nc.vector.tensor_scalar(y[:], y2[:], 1.0, None, op0=ALU.subtract)
                negc = scal_p.tile([P, 1], F32)
                nc.vector.tensor_scalar(negc[:], y[:], -1.0, None, op0=ALU.mult)
                return negc
            a0, a1, a2, a3 = LNC_A
            bits = scal_p.tile([P, 1], F32)
            nc.vector.tensor_copy(bits[:], S[:].bitcast(mybir.dt.int32))
            nb = scal_p.tile([P, 1], F32, tag="nb")
            nc.vector.tensor_scalar(
                nb[:], bits[:], -LN2 / (1 << 23), 127.0 * LN2,
                op0=ALU.mult, op1=ALU.add,
            )
            mb = scal_p.tile([P, 1], mybir.dt.int32, tag="mb")
            nc.vector.tensor_scalar(
                mb[:], S[:].bitcast(mybir.dt.int32), 0x007FFFFF, 0x3F800000,
                op0=ALU.bitwise_and, op1=ALU.bitwise_or,
            )
            m = mb[:].bitcast(F32)
            t = scal_p.tile([P, 1], F32, tag="h")
            nc.vector.tensor_scalar(t[:], m, -a3, -a2, op0=ALU.mult, op1=ALU.add)
            t2 = scal_p.tile([P, 1], F32, tag="h")
            nc.vector.tensor_tensor(t2[:], t[:], m, op=ALU.mult)
            t3 = scal_p.tile([P, 1], F32, tag="h")
            nc.vector.tensor_scalar(t3[:], t2[:], -a1, None, op0=ALU.add)
            t4 = scal_p.tile([P, 1], F32, tag="h")
            nc.vector.tensor_tensor(t4[:], t3[:], m, op=ALU.mult)
            t5 = scal_p.tile([P, 1], F32, tag="h")
            nc.vector.tensor_scalar(t5[:], t4[:], -a0, None, op0=ALU.add)
            negc = scal_p.tile([P, 1], F32)
            nc.vector.tensor_tensor(negc[:], nb[:], t5[:], op=ALU.add)
            return negc

        def emit_out(b, s_tiles, negc):
            for st in range(NST):
                for h in range(8):
                    og = ostage_p.tile([P, 1024], FP16)
                    nc.vector.tensor_scalar(
                        og[:],
                        s_tiles[st][:, 1024 * h : 1024 * (h + 1)],
                        -1.0,
                        negc[:],
                        op0=ALU.mult,
                        op1=ALU.add,
                    )
                    j0 = 8192 * st + 1024 * h
                    nc.sync.dma_start(
                        out=out_d[b * P : (b + 1) * P, j0 : j0 + 1024],
                        in_=og[:],
                    )

        # ---------------- phase 1: y prologue + block 0 (and b1 st0) ----------------
        # The first sqrts wait for the corr roundtrip (bhat group 0 store ->
        # ct load), so jg 0..3 are emitted as one batch at k==3; block 1's
        # first s-tile rides the same ct/yT deliveries (k < 8) to fill ACT.
        s0_tiles = []
        pending = [None]
        partials0 = part_p.tile([P, NST], F32)
        partials1 = part_p.tile([P, NST], F32)
        b1_st0 = None
        s_t = None
        for k in range(NSUP):
            if k == 0:
                # before the first y super: the rcorr rt copies land at the
                # head of the DVE queue (ahead of the y scales) so the Pool
                # stores' wait-at-head is short and ct(0) isn't starved
                emit_rcorr_chunk(0)
            emit_y_super(k)
            if k == STD1_K:
                emit_std_chunk(1)
            if k == 10:
                emit_rcorr_chunk(1)
            if k % 2 == 1:
                if not (JG0_FIRST and not EARLY_JG and k == 3):
                    emit_bhat_group((k - 1) // 2)
                if k == (1 if EARLY_JG else 3):
                    # after bhat group 0's DVE work so the corr roundtrip
                    # (which gates the first sqrt) isn't queued behind the
                    # x-side DVE chain
                    emit_x_side()
                if EARLY_JG and k == 1:
                    jgr = range(0, 2)
                elif k < 3:
                    continue
                elif k == 3:
                    jgr = range(0, 4) if not EARLY_JG else range(2, 4)
                else:
                    jgr = range(k - 1, k + 1)
                for jg in jgr:
                    if jg % 2 == 1:
                        continue
                    if JG0_FIRST and not EARLY_JG and k == 3 and jg == 2:
                        # ct(0) only needs bhat group 0: the jg 0-1 batch is
                        # emitted before group 1's stores so ct(0) isn't
                        # queued on Pool behind stores waiting on supers 2-3
                        emit_bhat_group(1)
                    ct = emit_ct_load(jg)
                    if jg % 8 == 0:
                        s_t = s_p.tile([P, 8192], FP16, tag="s_t")
                        s0_tiles.append(s_t)
                        if jg == 0:
                            # allocated after s0's first tile: pool rotation
                            # then lands block 1's later tiles on slots that
                            # free early
                            b1_st0 = s_p.tile([P, 8192], FP16, tag="s_t")
                    mms = emit_mm_pair(0, jg, ct)
                    emit_sqrt(s_t, jg % 8, mms[0])
                    emit_sqrt(s_t, jg % 8 + 1, mms[1])
                    if k < 8:
                        # block 1's first s-tile rides the same deliveries
                        mms1 = emit_mm_pair(1, jg, ct)
                        emit_sqrt(b1_st0, jg % 8, mms1[0])
                        emit_sqrt(b1_st0, jg % 8 + 1, mms1[1])
                if k == PENDCT_K:
                    # block 1's first steady ct: emitted mid-prologue so it
                    # sits AHEAD of the late cts (26..30) on the Pool queue;
                    # emitted at the back it transfers only after the k=31
                    # exp and block 1's corr matmuls restart cold
                    pending[0] = emit_ct_load(8)
                if PRO_EXPS == 1 and k in EXP_KS:
                    st_i = EXP_KS.index(k)
                    emit_exp(s0_tiles[st_i], partials0, st_i)
                    if st_i == 0:
                        emit_exp(b1_st0, partials1, 0)
                elif PRO_EXPS == 2 and k in (15, 31):
                    # pairs: one Exp-table residency per two s tiles
                    base = (k - 15) // 16 * 2
                    emit_exp(s0_tiles[base], partials0, base)
                    emit_exp(s0_tiles[base + 1], partials0, base + 1)
                    if k == 31:
                        emit_exp(b1_st0, partials1, 0)
        # block 1 resumes at jg 8; pre-load its corr tile while ACT runs the
        # phase-1 exp tail so the first steady-state sqrt isn't DMA-gated
        pending_ct = pending[0] if pending[0] is not None else emit_ct_load(8)
        # phase-1 PSUM (3 mm bufs + transpose tiles) closes here; steady
        # state gets a 4-deep mm pool so PE can pre-run a whole extra jg
        # pair across each block transition.  The SBUF staging pools close
        # too, freeing room for a 6th s-tile slot (sx_p) that decouples each
        # block's first fresh s tile from the previous block's out axpys.
        psum1.close()
        stage1.close()
        MM["ps"] = ctx.enter_context(
            tc.tile_pool(
                name="mmps2",
                bufs=2 if STEADY_WIDE else MM_BUFS2,
                space="PSUM",
            )
        )
        sx_p = ctx.enter_context(tc.tile_pool(name="sx", bufs=1, space="SBUF"))
        if not PRO_EXPS:
            # batched phase-1 exps: one Exp table residency instead of three
            # Sqrt<->Exp round trips inside the prologue
            for st in range(NST):
                emit_exp(s0_tiles[st], partials0, st)
            emit_exp(b1_st0, partials1, 0)
        negc0 = emit_lnS(partials0)
        emit_out(0, s0_tiles, negc0)

        # ---------------- blocks 1..NB-1 ----------------
        for b in range(1, NB):
            if b == 1:
                partials = partials1
                s_tiles = [b1_st0]
                st_range = range(1, NST)
            else:
                partials = part_p.tile([P, NST], F32)
                s_tiles = []
                st_range = range(NST)
            first_jg = 8 * st_range.start
            for st in st_range:
                sp = sx_p if (b, st) == SX_AT else s_p
                s_t = sp.tile([P, 8192], FP16, tag="s_t", name="s_t")
                for h2 in range(4):
                    jg = 8 * st + 2 * h2
                    ct = pending_ct if jg == first_jg else emit_ct_load(jg)
                    if STEADY_WIDE:
                        mm = emit_mm_quad(b, jg, ct)
                        emit_sqrt2(s_t, h2, mm)
                    else:
                        mms = emit_mm_pair(b, jg, ct)
                        emit_sqrt(s_t, 2 * h2, mms[0])
                        emit_sqrt(s_t, 2 * h2 + 1, mms[1])
                s_tiles.append(s_t)
            if b < NB - 1:
                # pre-load the next block's first corr tile during this
                # block's exp phase
                pending_ct = emit_ct_load(0)
            for st in st_range:
                emit_exp(s_tiles[st], partials, st)
            negc = emit_lnS(partials)
            emit_out(b, s_tiles, negc)

    nc.finalize()
    return nc


_NC_CACHE = {}


def _get_nc(rows, M):
    key = (rows, M)
    if key not in _NC_CACHE:
        _NC_CACHE[key] = build_nc(rows, M)
    return _NC_CACHE[key]


def kernel(x: np.ndarray, y: np.ndarray, std: np.ndarray) -> np.ndarray:
    x = np.ascontiguousarray(x, dtype=np.float32)
    y = np.ascontiguousarray(y, dtype=np.float32)
    std = np.ascontiguousarray(std, dtype=np.float32)
    N, M = x.shape[0], y.shape[0]
    rows = N // N_CORES
    nc = _get_nc(rows, M)
    in_maps = [
        {"x": x[c * rows : (c + 1) * rows], "y": y, "std": std}
        for c in range(N_CORES)
    ]
    trace = bool(int(os.environ.get("KERNEL_TRACE", "0")))
    res = run_bass_kernel_spmd(
        nc, in_maps, core_ids=list(range(N_CORES)), trace=trace
    )
    global LAST_RESULT
    LAST_RESULT = res
    return np.concatenate(
        [res.results[c]["out"] for c in range(N_CORES)], axis=0
    ).astype(np.float32)


LAST_RESULT = None



# revision 61
# speedup vs baseline: 1.0042x; 1.0004x over previous
"""Trainium2 Bass kernel for MatchingLayerL2:
   out = log_softmax(-sqrt(||x_i - y_j||^2) / std_j, axis=1)

x: [4096, 128] f32, y: [32768, 128] f32, std: [32768] f32 -> out [4096, 32768] f32.

Strategy: shard rows of x across 8 cores (512 rows each); y/std replicated.
Per core:
  rstd2_j = 1/std_j^2
  q_ij = rstd2_j * dist2_ij = (-2 x_i) . (y_j rstd2_j) + a_i rstd2_j + (b_j rstd2_j)
       (a = ||x||^2 rowwise, b = ||y hat||^2 * std^2 rowwise)
  s_ij = sqrt(q_ij) = dist_ij * rstd_j          (fp16 in SBUF)
  out_ij = -s_ij - ln(sum_j exp(-s_ij))          (no max-shift: s in [7,47])
Main matmul in bf16 (K=128); the rank-2 correction a*r + b*r is added with a
K=5 bf16 matmul whose rows are hi/lo bf16 splits for fp32-grade accuracy.
The 5 correction rows are staged through an internal DRAM tensor because a
[5, M] SBUF tile would charge M*2 bytes across all 128 partitions.

Schedule notes (tuned against the TimelineSim cost model; every feature
device-verified -- tensor_tensor_reduce and Pool tensor_scalar crash/wedge
the NeuronCore runtime and must not be used):
 - ACT (scalar) is the bottleneck: sqrt-from-PSUM and exp-with-accum are both
   mandatory full sweeps (~0.83 ns/elem each).  Everything else is kept under
   it: output written fp16 (halves DMA + runs the final axpy at DVE 4x),
   b-hat via one fused affine_mul_reduce per 128-column, half the yT staging
   copies on otherwise-idle ACT (Copy lives in every table set), -ln(S)
   entirely on DVE (bitcast exponent seed + cubic minimax for
   ln(m)-ln2*(m-1), |err|<=1e-3) so no table load or Newton exp sits
   between a block's last exp and the next block's Sqrt table load.
 - Engine SEQs are in-order and DMA instructions hold their queue while
   waiting, so program order ~= queue order.  Block 0's sqrt/exp (plus block
   1's first s-tile) and the corr loads are interleaved with the y-prologue
   super-chunks; rcorr chunk 0 is emitted before the first y super so its rt
   copies sit at the head of the DVE queue and ct(0) isn't starved.
 - PE matmul speed is decided at SEQ *visit* time from the length of PE's
   current busy-run (>3us full, >100ns mid, else low).  A SEQ blocked on a
   long wait re-visits with PE freshly restarted -> 788ns matmuls.  The
   phase-1 PSUM pools (3 mm bufs + transpose tiles) are closed after the
   prologue and steady state gets a 4-deep mm pool: the 4-tile runway of
   pre-computed matmuls absorbs the post-exp p-state ramp at each block
   transition.
 - SBUF is the binding constraint (~207.9 KB/partition).  Phase-1 staging
   pools live in a nested stack created last and closed after the prologue;
   the freed space hosts a 6th s-tile slot (sx_p, taken by block 1's st1)
   which decouples each block's first fresh s tile from the previous
   block's DMA-throttled out axpys (the out stream takes ~5.8us/s-tile).
 - The ~24us tail (last block's 8.4MB fp16 output after its logsumexp
   barrier) is DMA-bandwidth floor; the softmax normalizer makes it
   irreducible without approximating lnS.
"""

import os
import sys

sys.path.insert(0, "/root/.axon_site/_ro/trn_rl_repo")

import numpy as np
from contextlib import ExitStack

import concourse.bass as bass
from concourse import bacc
import concourse.tile as tile
from concourse.tile import add_dep_helper
from concourse import mybir, masks
from concourse.bass_utils import run_bass_kernel_spmd

F32 = mybir.dt.float32
BF16 = mybir.dt.bfloat16
FP16 = mybir.dt.float16
AF = mybir.ActivationFunctionType
ALU = mybir.AluOpType
AX = mybir.AxisListType

N_CORES = 8
D = 128
P = 128
# scheduling knobs (tuned against TimelineSim)
YT_MODE = 1   # yt copies: 0/1 alternate (ACT on that parity), 2 all DVE, 3 all ACT
YLOAD_SPLIT = 3   # split each y super-chunk load into (v+1) pieces
MM_BUFS = 3   # psum matmul tiles (2 banks each)
YSTAGE_BUFS = 2
OSTAGE_BUFS = 5
YBAR_BUFS = 2
ROWT_BUFS = 2
B1_EXP_AT7 = True
EARLY_JG = False
RC_ON_POOL = True
STDX_ON_POOL = False
NEWTON_ITERS = 2
PRO_EXPS = True   # emit phase-1 exps inside the prologue (False = batch after)
YT_FORCE_DVE = ()  # yt copies forced to DVE (tested: parity as-is wins)
CORRT_BUFS = 3
SX_AT = (1, 1)  # which steady s tile takes the extra 6th slot
JG0_FIRST = False  # tested: reordering jg0 before bhat g1 is worse
Y_ACT_SUPERS = 0  # tested: y loads via ACT hwdge queue are worse
STEADY_WIDE = False  # tested: 2-deep 2048 runway loses to 4-deep 1024
PIN_SQRT_TABLE = True  # dummy sqrt pins the initial act table to the Sqrt set
YB_INPLACE = False  # tested: in-place yb scale couples staging, loses 3.2us
YSTAGE_IP_BUFS = 3
PENDCT_K = 99  # prologue k at which to pre-emit block 1's ct(8); 99 = after loop
EXP_KS = (7, 15, 23, 31)  # prologue supers at which b0's s-tile exps run
ES_FP8 = True  # exp scratch output is discarded; fp8 halves its SBUF charge
BHAT_ONE_STORE = False  # tested: combined bhat store holds tp longer, +3.4us
STD1_K = -1  # std chunk 1 upfront (deferred emission tested worse)
RC0_K = 0  # super at which rcorr chunk 0 is emitted (0 = before first super)
XSIDE_K = 0  # 0 = default (k==3 for EARLY_JG=False); must be <= first-mm k
CT8_ON_SP = True  # block 1's preloaded ct(8) via the idle SP queue
TP_BUFS = 2   # psum transpose tiles (1 bank each); 2*MM_BUFS + TP_BUFS <= 8
MM_BUFS2 = 4  # steady-state psum matmul tiles (phase-1 psum pools are closed
              # first, freeing their banks); 2*MM_BUFS2 <= 8
NEGC_DVE = True  # -ln(S) fully on DVE (bit trick + cubic); keeps the Exp->Sqrt
                 # table load off the block-transition critical path
ST_ON_DVE = False  # DVE is not an hwdge engine on TRN2; stores stay on Pool
# c(m) = ln(m) - ln2*(m-1) on [1,2): minimax cubic, |err| <= 9.3e-4
LNC_A = (-0.78590174, 1.39373203, -0.71359, 0.10668473)


def build_nc(rows, M):
    """Build the Bass module for one core: x shard [rows, D], y [M, D], std [M]."""
    NB = rows // P          # row blocks of 128
    NSUP = M // 1024        # y super-chunks (1024 rows each) == 1024-col groups
    NST = M // 8192         # s tiles per block (8192 cols each)
    nA = M // P             # layout-A columns: v[q, t] = v[t*128 + q]

    nc = bacc.Bacc("TRN2", target_bir_lowering=False, debug=False, num_swdge_queues=4)
    x_d = nc.declare_dram_parameter("x", [rows, D], F32, isOutput=False)
    y_d = nc.declare_dram_parameter("y", [M, D], F32, isOutput=False)
    std_d = nc.declare_dram_parameter("std", [M], F32, isOutput=False)
    out_d = nc.declare_dram_parameter("out", [rows, M], FP16, isOutput=True)
    corr_d = nc.dram_tensor("corr", [5, M], BF16, kind="Internal")

    act_prev = [None]

    def act(*a, **k):
        inst = nc.scalar.activation(*a, **k)
        if act_prev[0] is not None:
            add_dep_helper(inst.ins, act_prev[0].ins, sync=False, reason="act order")
        act_prev[0] = inst
        return inst

    with tile.TileContext(nc) as tc, ExitStack() as ctx:
        pool = lambda name, bufs, space="SBUF": ctx.enter_context(
            tc.tile_pool(name=name, bufs=bufs, space=space)
        )

        # Persistent pools first; phase-1 staging pools go in their own stack
        # (created last = top of the SBUF stack) so closing them after the y
        # prologue frees real address space for the extra steady-state s slot.
        const_p = pool("const", 1)
        yT_p = pool("yT", NSUP)         # 32 x [128, 1024] bf16
        lhs_p = pool("lhs", 1)
        lhsc_p = pool("lhsc", NB)
        corrt_p = pool("corrt", CORRT_BUFS)
        s_p = pool("s", NST + 1)        # 5 x [128, 8192] fp16
        part_p = pool("part", 2)
        scal_p = pool("scal", 6)
        escr_p = pool("escr", 1)
        ostage_p = pool("ostage", OSTAGE_BUFS)  # [128, 1024] fp16

        stage1 = ctx.enter_context(ExitStack())
        spool = lambda name, bufs: stage1.enter_context(
            tc.tile_pool(name=name, bufs=bufs, space="SBUF")
        )
        ystage_p = spool("ystage", YSTAGE_IP_BUFS if YB_INPLACE else YSTAGE_BUFS)
        if not YB_INPLACE:
            ybar_p = spool("ybar", YBAR_BUFS)
        sq2_p = spool("sq2", 2)         # TTR product scratch [128, 128] bf16
        colsA_p = spool("colsA", 1)     # stdA, rstdA, rA, std2A  (f32 [128, nA])
        bcols_p = spool("bcols", 1)     # b2A f32 [128, nA]
        bg_p = spool("bg", 2)           # per-group bhat tiles [128, 16]
        rowT_p = spool("rowT", ROWT_BUFS)       # transposed row chunks [*, 128] bf16
        xa_p = spool("xa", 1)
        acol_p = spool("acol", 1)

        # phase-1 PSUM pools live in their own stack: closed after the y
        # prologue so the steady-state pool can take all 8 banks (deeper PE
        # runway over the block transition hides the p-state ramp).
        psum1 = ctx.enter_context(ExitStack())
        mm_ps = psum1.enter_context(
            tc.tile_pool(name="mmps", bufs=MM_BUFS, space="PSUM")
        )  # [128,1024] f32 = 2 banks each
        tp_ps = psum1.enter_context(
            tc.tile_pool(name="tpps", bufs=TP_BUFS, space="PSUM")
        )  # [128,1024] bf16 = 1 bank each
        MM = {"ps": mm_ps}

        # ---------------- constants ----------------
        ident = const_p.tile([P, P], BF16)
        masks.make_identity(nc, ident[:])
        identf = const_p.tile([P, P], F32)
        masks.make_identity(nc, identf[:])
        if PIN_SQRT_TABLE:
            # the first real ACT op is a Copy (in every table set); without a
            # hint the initial table load picks a non-Sqrt set and a second
            # 1283ns load lands on the startup critical path before the
            # first sqrt.  A 1-element dummy Sqrt pins the right set.
            tpin = scal_p.tile([P, 1], F32, tag="h")
            act(tpin[:], identf[:, 0:1], AF.Sqrt)

        # ---------------- std-derived quantities (layout A) ----------------
        # Minimal chain to rA first: the y prologue (ybar scaling) needs it.
        # stdA[q, t] = std[128 t + q]: load natural [t, q] tiles, PE-transpose.
        stdA = colsA_p.tile([P, nA], F32)
        std2A = colsA_p.tile([P, nA], F32)
        rstdA = colsA_p.tile([P, nA], F32)
        rA = colsA_p.tile([P, nA], F32)
        stn_tiles = []

        def emit_std_chunk(c):
            # per-chunk std->rA chain: chunk 1 is deferred past the first y
            # supers (its rA cols feed supers >= 16; its stn feeds rcorr
            # chunk 1 at k==10) so it doesn't serialize ahead of y0/y1 on SP
            h = min(P, nA - c * P)
            stn = rowT_p.tile([P, P], F32, tag="stn")
            (nc.gpsimd if STDX_ON_POOL else nc.sync).dma_start(
                out=stn[0:h, :],
                in_=std_d[P * P * c : P * (P * c + h)].rearrange(
                    "(t q) -> t q", q=P
                ),
            )
            stn_tiles.append(stn)
            tpf = tp_ps.tile([P, P], F32, tag="tp")
            nc.tensor.transpose(tpf[:, 0:h], stn[0:h, :], identf[:])
            csl = slice(c * P, c * P + h)
            nc.vector.tensor_copy(stdA[:, csl], tpf[:, 0:h])
            nc.vector.tensor_tensor(
                std2A[:, csl], stdA[:, csl], stdA[:, csl], op=ALU.mult
            )
            nc.vector.reciprocal(rstdA[:, csl], stdA[:, csl])
            nc.vector.tensor_tensor(
                rA[:, csl], rstdA[:, csl], rstdA[:, csl], op=ALU.mult
            )

        emit_std_chunk(0)
        if STD1_K < 0:
            emit_std_chunk(1)
        # corr rows 0,1 = r_hi (pairs with a_hi, a_lo), row 2 = r_lo (pairs
        # a_hi).  Computed DIRECTLY on the natural-layout stn tiles (their
        # [t, q] order IS corr_d's row-major j order), so the stores depend
        # only on the std load -- not on the stdA-transpose/rA chain that
        # backs up the DVE queue at startup.
        def emit_rcorr_chunk(c):
            w = min(P, nA - c * P)
            stn = stn_tiles[c]
            rn = rowT_p.tile([P, P], F32, tag="rn")
            nc.vector.reciprocal(rn[0:w, :], stn[0:w, :])
            rq = rowT_p.tile([P, P], F32, tag="rq")
            nc.vector.tensor_tensor(rq[0:w, :], rn[0:w, :], rn[0:w, :], op=ALU.mult)
            rhi_n = rowT_p.tile([P, P], BF16, tag="rowT")
            nc.vector.tensor_copy(rhi_n[0:w, :], rq[0:w, :])
            rlo_n = rowT_p.tile([P, P], BF16, tag="rowT")
            nc.vector.tensor_tensor(
                rlo_n[0:w, :], rq[0:w, :], rhi_n[0:w, :], op=ALU.subtract
            )
            eng = nc.gpsimd if RC_ON_POOL else nc.sync
            for row, srcn in ((0, rhi_n), (1, rhi_n), (2, rlo_n)):
                eng.dma_start(
                    out=corr_d[row, c * P * P : (c * P + w) * P].rearrange(
                        "(t q) -> t q", q=P
                    ),
                    in_=srcn[0:w, :],
                )

        # ---------------- x side (emitted at phase-1 k==2) ----------------
        # lhsT_main = (-2x)^T bf16, a = ||x||^2.  Deferred into the super-chunk
        # loop so the first y super-chunks' DVE work isn't queued behind it
        # (engine queues are in-order); it's only needed by the first matmul.
        lhsT_main = lhs_p.tile([P, rows], BF16)
        lhsT_corr = []

        def emit_x_side():
            xstage = xa_p.tile([P, NB, D], F32)
            (nc.gpsimd if STDX_ON_POOL else nc.sync).dma_start(
                out=xstage[:], in_=x_d[:, :].rearrange("(c p) d -> p c d", p=P)
            )
            a_cols = acol_p.tile([P, NB], F32)
            for c in range(NB):
                xs2 = sq2_p.tile([P, D], BF16, tag="xs2")
                nc.vector.affine_mul_reduce(
                    out=xs2[:],
                    accum_out=a_cols[:, c : c + 1],
                    in0=xstage[:, c, :],
                    in1=xstage[:, c, :],
                    scale=1.0,
                    bias=0.0,
                )
            ahi_col = acol_p.tile([P, NB], BF16)
            nc.vector.tensor_copy(ahi_col[:], a_cols[:])
            alo_col = acol_p.tile([P, NB], BF16)
            nc.vector.tensor_tensor(alo_col[:], a_cols[:], ahi_col[:], op=ALU.subtract)

            xbar = xa_p.tile([P, NB, D], BF16, tag="xbar")
            nc.vector.tensor_scalar(xbar[:], xstage[:], -2.0, None, op0=ALU.mult)
            for c in range(NB):
                tp = tp_ps.tile([P, 1024], BF16, tag="tp")
                nc.tensor.transpose(tp[:, 0:P], xbar[:, c, :], ident[:])
                nc.vector.tensor_copy(lhsT_main[:, c * P : (c + 1) * P], tp[:, 0:P])

            # lhsT_corr per block: rows [a_hi; a_lo; a_hi; 1; 1] as [5, 128] bf16
            for b in range(NB):
                asm = acol_p.tile([P, 8], BF16, tag="asm")
                nc.vector.tensor_copy(asm[:, 0:1], ahi_col[:, b : b + 1])
                nc.vector.tensor_copy(asm[:, 1:2], alo_col[:, b : b + 1])
                nc.vector.tensor_copy(asm[:, 2:3], ahi_col[:, b : b + 1])
                nc.vector.memset(asm[:, 3:5], 1.0)
                tp = tp_ps.tile([P, 1024], BF16, tag="tp")
                nc.tensor.transpose(tp[0:5, 0:P], asm[:, 0:5], ident[:])
                lc = lhsc_p.tile([5, P], BF16)
                nc.vector.tensor_copy(lc[:], tp[0:5, 0:P])
                lhsT_corr.append(lc)

        # ---------------- shared emitters ----------------
        yT = []                 # 32 x [128, 1024] bf16 (super-chunk k)
        b2A = bcols_p.tile([P, nA], F32)

        def emit_y_super(k):
            """Load+scale+transpose y rows [1024k, 1024(k+1)); fill b-hat cols.

            b-hat comes from one fused affine_mul_reduce per 128-column:
            out = (yb * std2) * yb, accum = std2 * sum(yb^2) = ||yhat||^2 std^2.
            The yT staging copies alternate ACT/DVE: ACT has idle in phase 1
            (delivery-paced) and Copy lives in every activation table set."""
            yst = ystage_p.tile([P, 8, D], F32)
            # first supers ride the otherwise-idle ACT hwdge queue so the SP
            # queue (std + x) doesn't serialize the startup's y deliveries
            yq = nc.scalar if k < Y_ACT_SUPERS else nc.sync
            if YLOAD_SPLIT:
                # split loads: the ybar chain starts on the first piece
                # while the rest are still in flight
                nsp = YLOAD_SPLIT + 1
                assert 8 % nsp == 0, "YLOAD_SPLIT+1 must divide 8"
                cw = 8 // nsp
                for hh in range(nsp):
                    yq.dma_start(
                        out=yst[:, cw * hh : cw * (hh + 1), :],
                        in_=y_d[1024 * k + 128 * cw * hh : 1024 * k + 128 * cw * (hh + 1), :
                                ].rearrange("(c p) d -> p c d", p=P),
                    )
            else:
                yq.dma_start(
                    out=yst[:],
                    in_=y_d[1024 * k : 1024 * (k + 1), :].rearrange(
                        "(c p) d -> p c d", p=P
                    ),
                )
            if YB_INPLACE:
                # bf16 scale output written over the f32 stage bytes it just
                # read (per chunk: write bytes [512c,512c+256) trail the read
                # of [512c,512c+512)) -- the ybar staging pool disappears and
                # its 4KB funds a deeper ystage
                yb = yst[:].bitcast(BF16)[:, :, 0:D]
            else:
                yb = ybar_p.tile([P, 8, D], BF16)
            for c in range(8):
                nc.vector.tensor_scalar(
                    yb[:, c, :],
                    yst[:, c, :],
                    rA[:, 8 * k + c : 8 * k + c + 1],
                    None,
                    op0=ALU.mult,
                )
            for c in range(8):
                sq2 = sq2_p.tile([P, D], BF16)
                nc.vector.affine_mul_reduce(
                    out=sq2[:],
                    accum_out=b2A[:, 8 * k + c : 8 * k + c + 1],
                    in0=yb[:, c, :],
                    in1=yb[:, c, :],
                    scale=std2A[:, 8 * k + c : 8 * k + c + 1],
                    bias=0.0,
                )
            tp = tp_ps.tile([P, 1024], BF16, tag="tp")
            for c in range(8):
                nc.tensor.transpose(
                    tp[:, c * P : (c + 1) * P], yb[:, c, :], ident[:]
                )
            yt = yT_p.tile([P, 1024], BF16)
            # YT_MODE: 0/1 = alternate (ACT on that parity), 2 = all DVE,
            # 3 = all ACT.  ACT copies stay unchained: Copy lives in every
            # activation table set, and chaining would lock ACT progress to
            # y-super delivery.
            on_act = (YT_MODE == 3) or (YT_MODE in (0, 1) and k % 2 == YT_MODE)
            if k in YT_FORCE_DVE:
                # supers consumed right after a prologue exp: an ACT-queued
                # copy would sit behind the 7us exp and stall the next mms
                on_act = False
            if on_act:
                nc.scalar.copy(yt[:], tp[:])
            else:
                nc.vector.tensor_copy(yt[:], tp[:])
            yT.append(yt)

        def emit_bhat_group(g):
            """b-hat hi/lo rows for layout-A cols [16g, 16(g+1)) -> corr_d."""
            csl = slice(16 * g, 16 * (g + 1))
            bhi = bg_p.tile([P, 16], BF16, tag="bhi")
            nc.vector.tensor_copy(bhi[:], b2A[:, csl])
            blo = bg_p.tile([P, 16], BF16, tag="blo")
            nc.vector.tensor_tensor(blo[:], b2A[:, csl], bhi[:], op=ALU.subtract)
            if BHAT_ONE_STORE:
                # both rows stacked in one [32,128] tile -> a single DMA
                # (rows 3 and 4 of corr_d via a 2D pattern): halves the Pool
                # gen count feeding every ct load
                tp2 = tp_ps.tile([P, 1024], BF16, tag="tp")
                nc.tensor.transpose(tp2[0:16, 0:P], bhi[:], ident[:])
                # PE transpose writes need base partition 0/32/64
                nc.tensor.transpose(tp2[32:48, 0:P], blo[:], ident[:])
                rt = rowT_p.tile([P, P], BF16, tag="rowT")
                nc.vector.tensor_copy(rt[0:16, :], tp2[0:16, 0:P])
                nc.vector.tensor_copy(rt[16:32, :], tp2[32:48, 0:P])
                nc.gpsimd.dma_start(
                    out=corr_d[3:5, 2048 * g : 2048 * (g + 1)].rearrange(
                        "r (t q) -> r t q", q=P
                    ),
                    in_=rt[0:32, :].rearrange("(r t) q -> r t q", r=2),
                )
            else:
                for row, src in ((3, bhi), (4, blo)):
                    tp2 = tp_ps.tile([P, 1024], BF16, tag="tp")
                    nc.tensor.transpose(tp2[0:16, 0:P], src[:], ident[:])
                    rt = rowT_p.tile([P, P], BF16, tag="rowT")
                    nc.vector.tensor_copy(rt[0:16, :], tp2[0:16, 0:P])
                    (nc.vector if ST_ON_DVE else nc.gpsimd).dma_start(
                        out=corr_d[row, 2048 * g : 2048 * (g + 1)].rearrange(
                            "(t q) -> t q", q=P
                        ),
                        in_=rt[0:16, :],
                    )

        def emit_ct_load(jg, eng=None):
            """corr rows for cols [1024 jg, 1024 (jg+2)); jg even."""
            ct = corrt_p.tile([5, 2048], BF16)
            (eng or nc.gpsimd).dma_start(
                out=ct[:], in_=corr_d[:, 1024 * jg : 1024 * (jg + 2)]
            )
            return ct

        def emit_mm_pair(b, jg0, ct):
            """Main+corr matmuls for col groups jg0, jg0+1 of block b.
            The ISA caps a matmul's moving free dim at 512, so each 1024-col
            group is two matmuls; all four mains run back-to-back before the
            four corrs so the stationary tensor only switches once per pair."""
            mms = []
            for i in range(2):
                mm = MM["ps"].tile([P, 1024], F32, tag="mm")
                for q in range(2):
                    nc.tensor.matmul(
                        mm[:, 512 * q : 512 * (q + 1)],
                        lhsT_main[:, b * P : (b + 1) * P],
                        yT[jg0 + i][:, 512 * q : 512 * (q + 1)],
                        start=True,
                        stop=False,
                    )
                mms.append(mm)
            for i in range(2):
                for q in range(2):
                    nc.tensor.matmul(
                        mms[i][:, 512 * q : 512 * (q + 1)],
                        lhsT_corr[b][:],
                        ct[:, 1024 * i + 512 * q : 1024 * i + 512 * (q + 1)],
                        start=False,
                        stop=True,
                    )
            return mms

        def emit_mm_quad(b, jg0, ct):
            """Steady-state variant: one [128, 2048] psum tile covering col
            groups jg0, jg0+1 (exactly one ct tile), consumed by ONE
            2048-wide sqrt -- halves the per-sqrt ACT access-latency
            charge."""
            mm = MM["ps"].tile([P, 2048], F32, tag="mm")
            for i in range(4):
                nc.tensor.matmul(
                    mm[:, 512 * i : 512 * (i + 1)],
                    lhsT_main[:, b * P : (b + 1) * P],
                    yT[jg0 + i // 2][:, 512 * (i % 2) : 512 * (i % 2 + 1)],
                    start=True,
                    stop=False,
                )
            for i in range(4):
                nc.tensor.matmul(
                    mm[:, 512 * i : 512 * (i + 1)],
                    lhsT_corr[b][:],
                    ct[:, 512 * i : 512 * (i + 1)],
                    start=False,
                    stop=True,
                )
            return mm

        def emit_sqrt(s_t, h, mm):
            act(s_t[:, 1024 * h : 1024 * (h + 1)], mm[:], AF.Sqrt)

        def emit_sqrt2(s_t, h2, mm):
            act(s_t[:, 2048 * h2 : 2048 * (h2 + 1)], mm[:], AF.Sqrt)

        def emit_exp(s_t, partials, st):
            es = escr_p.tile([P, 8192], mybir.dt.float8e4 if ES_FP8 else BF16)
            act(
                es[:],
                s_t[:],
                AF.Exp,
                scale=-1.0,
                accum_out=partials[:, st : st + 1],
            )

        LN2 = 0.6931471805599453

        def emit_lnS(partials):
            # negc = -ln(S), entirely on DVE so no Ln/Exp table dependency:
            # S = m 2^E; bits/2^23 = (E+127) + (m-1);
            # ln S = ln2*(bits/2^23 - 127) + c(m), c = ln m - ln2 (m-1)
            # (cubic minimax for c, |err| <= 9.3e-4 -> ~5e-5 rel on output).
            S = scal_p.tile([P, 1], F32)
            nc.vector.tensor_reduce(S[:], partials[:], axis=AX.X, op=ALU.add)
            if not NEGC_DVE:
                bits = scal_p.tile([P, 1], F32)
                nc.vector.tensor_copy(bits[:], S[:].bitcast(mybir.dt.int32))
                y = scal_p.tile([P, 1], F32, tag="y")
                nc.vector.tensor_scalar(
                    y[:], bits[:], LN2 / (1 << 23), -(127.0 - 0.043) * LN2,
                    op0=ALU.mult, op1=ALU.add,
                )
                for _ in range(NEWTON_ITERS):
                    e = scal_p.tile([P, 1], F32, tag="nwe")
                    act(e[:], y[:], AF.Exp, scale=-1.0)
                    t = scal_p.tile([P, 1], F32, tag="nwt")
                    nc.vector.tensor_tensor(t[:], e[:], S[:], op=ALU.mult)
                    y2 = scal_p.tile([P, 1], F32, tag="y")
                    nc.vector.tensor_tensor(y2[:], y[:], t[:], op=ALU.add)
                    y = scal_p.tile([P, 1], F32, tag="y")
                    nc.vector.tensor_scalar(y[:], y2[:], 1.0, None, op0=ALU.subtract)
                negc = scal_p.tile([P, 1], F32)
                nc.vector.tensor_scalar(negc[:], y[:], -1.0, None, op0=ALU.mult)
                return negc
            a0, a1, a2, a3 = LNC_A
            bits = scal_p.tile([P, 1], F32)
            nc.vector.tensor_copy(bits[:], S[:].bitcast(mybir.dt.int32))
            nb = scal_p.tile([P, 1], F32, tag="nb")
            nc.vector.tensor_scalar(
                nb[:], bits[:], -LN2 / (1 << 23), 127.0 * LN2,
                op0=ALU.mult, op1=ALU.add,
            )
            mb = scal_p.tile([P, 1], mybir.dt.int32, tag="mb")
            nc.vector.tensor_scalar(
                mb[:], S[:].bitcast(mybir.dt.int32), 0x007FFFFF, 0x3F800000,
                op0=ALU.bitwise_and, op1=ALU.bitwise_or,
            )
            m = mb[:].bitcast(F32)
            t = scal_p.tile([P, 1], F32, tag="h")
            nc.vector.tensor_scalar(t[:], m, -a3, -a2, op0=ALU.mult, op1=ALU.add)
            t2 = scal_p.tile([P, 1], F32, tag="h")
            nc.vector.tensor_tensor(t2[:], t[:], m, op=ALU.mult)
            t3 = scal_p.tile([P, 1], F32, tag="h")
            nc.vector.tensor_scalar(t3[:], t2[:], -a1, None, op0=ALU.add)
            t4 = scal_p.tile([P, 1], F32, tag="h")
            nc.vector.tensor_tensor(t4[:], t3[:], m, op=ALU.mult)
            t5 = scal_p.tile([P, 1], F32, tag="h")
            nc.vector.tensor_scalar(t5[:], t4[:], -a0, None, op0=ALU.add)
            negc = scal_p.tile([P, 1], F32)
            nc.vector.tensor_tensor(negc[:], nb[:], t5[:], op=ALU.add)
            return negc

        def emit_out(b, s_tiles, negc):
            for st in range(NST):
                for h in range(8):
                    og = ostage_p.tile([P, 1024], FP16)
                    nc.vector.tensor_scalar(
                        og[:],
                        s_tiles[st][:, 1024 * h : 1024 * (h + 1)],
                        -1.0,
                        negc[:],
                        op0=ALU.mult,
                        op1=ALU.add,
                    )
                    j0 = 8192 * st + 1024 * h
                    nc.sync.dma_start(
                        out=out_d[b * P : (b + 1) * P, j0 : j0 + 1024],
                        in_=og[:],
                    )

        # ---------------- phase 1: y prologue + block 0 (and b1 st0) ----------------
        # The first sqrts wait for the corr roundtrip (bhat group 0 store ->
        # ct load), so jg 0..3 are emitted as one batch at k==3; block 1's
        # first s-tile rides the same ct/yT deliveries (k < 8) to fill ACT.
        s0_tiles = []
        pending = [None]
        partials0 = part_p.tile([P, NST], F32)
        partials1 = part_p.tile([P, NST], F32)
        b1_st0 = None
        s_t = None
        for k in range(NSUP):
            if k == 0 and RC0_K == 0:
                # before the first y super: the rcorr rt copies land at the
                # head of the DVE queue (ahead of the y scales) so the Pool
                # stores' wait-at-head is short and ct(0) isn't starved
                emit_rcorr_chunk(0)
            emit_y_super(k)
            if k == STD1_K:
                emit_std_chunk(1)
            if k == 10:
                emit_rcorr_chunk(1)
            if k % 2 == 1:
                if not (JG0_FIRST and not EARLY_JG and k == 3):
                    emit_bhat_group((k - 1) // 2)
                if k == RC0_K:
                    # rcorr chunk 0 has ~8us of slack before ct(0); emitting
                    # it after the bhat-critical scales/amrs tests whether
                    # the DVE queue head is better spent on the bhat chain
                    emit_rcorr_chunk(0)
                if k == (XSIDE_K if XSIDE_K else (1 if EARLY_JG else 3)):
                    # after bhat group 0's DVE work so the corr roundtrip
                    # (which gates the first sqrt) isn't queued behind the
                    # x-side DVE chain
                    emit_x_side()
                if EARLY_JG and k == 1:
                    jgr = range(0, 2)
                elif k < 3:
                    continue
                elif k == 3:
                    jgr = range(0, 4) if not EARLY_JG else range(2, 4)
                else:
                    jgr = range(k - 1, k + 1)
                for jg in jgr:
                    if jg % 2 == 1:
                        continue
                    if JG0_FIRST and not EARLY_JG and k == 3 and jg == 2:
                        # ct(0) only needs bhat group 0: the jg 0-1 batch is
                        # emitted before group 1's stores so ct(0) isn't
                        # queued on Pool behind stores waiting on supers 2-3
                        emit_bhat_group(1)
                    ct = emit_ct_load(jg)
                    if jg % 8 == 0:
                        s_t = s_p.tile([P, 8192], FP16, tag="s_t")
                        s0_tiles.append(s_t)
                        if jg == 0:
                            # allocated after s0's first tile: pool rotation
                            # then lands block 1's later tiles on slots that
                            # free early
                            b1_st0 = s_p.tile([P, 8192], FP16, tag="s_t")
                    mms = emit_mm_pair(0, jg, ct)
                    emit_sqrt(s_t, jg % 8, mms[0])
                    emit_sqrt(s_t, jg % 8 + 1, mms[1])
                    if k < 8:
                        # block 1's first s-tile rides the same deliveries
                        mms1 = emit_mm_pair(1, jg, ct)
                        emit_sqrt(b1_st0, jg % 8, mms1[0])
                        emit_sqrt(b1_st0, jg % 8 + 1, mms1[1])
                if k == PENDCT_K:
                    # block 1's first steady ct: emitted mid-prologue so it
                    # sits AHEAD of the late cts (26..30) on the Pool queue;
                    # emitted at the back it transfers only after the k=31
                    # exp and block 1's corr matmuls restart cold
                    pending[0] = emit_ct_load(8)
                if PRO_EXPS == 1 and k in EXP_KS:
                    st_i = EXP_KS.index(k)
                    emit_exp(s0_tiles[st_i], partials0, st_i)
                    if st_i == 0:
                        emit_exp(b1_st0, partials1, 0)
                elif PRO_EXPS == 2 and k in (15, 31):
                    # pairs: one Exp-table residency per two s tiles
                    base = (k - 15) // 16 * 2
                    emit_exp(s0_tiles[base], partials0, base)
                    emit_exp(s0_tiles[base + 1], partials0, base + 1)
                    if k == 31:
                        emit_exp(b1_st0, partials1, 0)
        # block 1 resumes at jg 8; pre-load its corr tile while ACT runs the
        # phase-1 exp tail so the first steady-state sqrt isn't DMA-gated
        # ct(8) rides the now-idle SP queue: on Pool it sits behind the
        # head-of-line slot-waits of ct(26..30) and transfers only after the
        # k=31 exp, restarting block 1's corr matmuls cold
        pending_ct = (
            pending[0]
            if pending[0] is not None
            else emit_ct_load(8, eng=nc.sync if CT8_ON_SP else None)
        )
        # phase-1 PSUM (3 mm bufs + transpose tiles) closes here; steady
        # state gets a 4-deep mm pool so PE can pre-run a whole extra jg
        # pair across each block transition.  The SBUF staging pools close
        # too, freeing room for a 6th s-tile slot (sx_p) that decouples each
        # block's first fresh s tile from the previous block's out axpys.
        psum1.close()
        stage1.close()
        MM["ps"] = ctx.enter_context(
            tc.tile_pool(
                name="mmps2",
                bufs=2 if STEADY_WIDE else MM_BUFS2,
                space="PSUM",
            )
        )
        sx_p = ctx.enter_context(tc.tile_pool(name="sx", bufs=1, space="SBUF"))
        if not PRO_EXPS:
            # batched phase-1 exps: one Exp table residency instead of three
            # Sqrt<->Exp round trips inside the prologue
            for st in range(NST):
                emit_exp(s0_tiles[st], partials0, st)
            emit_exp(b1_st0, partials1, 0)
        negc0 = emit_lnS(partials0)
        emit_out(0, s0_tiles, negc0)

        # ---------------- blocks 1..NB-1 ----------------
        for b in range(1, NB):
            if b == 1:
                partials = partials1
                s_tiles = [b1_st0]
                st_range = range(1, NST)
            else:
                partials = part_p.tile([P, NST], F32)
                s_tiles = []
                st_range = range(NST)
            first_jg = 8 * st_range.start
            for st in st_range:
                sp = sx_p if (b, st) == SX_AT else s_p
                s_t = sp.tile([P, 8192], FP16, tag="s_t", name="s_t")
                for h2 in range(4):
                    jg = 8 * st + 2 * h2
                    ct = pending_ct if jg == first_jg else emit_ct_load(jg)
                    if STEADY_WIDE:
                        mm = emit_mm_quad(b, jg, ct)
                        emit_sqrt2(s_t, h2, mm)
                    else:
                        mms = emit_mm_pair(b, jg, ct)
                        emit_sqrt(s_t, 2 * h2, mms[0])
                        emit_sqrt(s_t, 2 * h2 + 1, mms[1])
                s_tiles.append(s_t)
            if b < NB - 1:
                # pre-load the next block's first corr tile during this
                # block's exp phase (SP queue: past out(b-1)'s gens, clear of
                # the Pool queue's tail-of-block slot-waits)
                pending_ct = emit_ct_load(0, eng=nc.sync if CT8_ON_SP else None)
            for st in st_range:
                emit_exp(s_tiles[st], partials, st)
            negc = emit_lnS(partials)
            emit_out(b, s_tiles, negc)

    nc.finalize()
    return nc


_NC_CACHE = {}


def _get_nc(rows, M):
    key = (rows, M)
    if key not in _NC_CACHE:
        _NC_CACHE[key] = build_nc(rows, M)
    return _NC_CACHE[key]


def kernel(x: np.ndarray, y: np.ndarray, std: np.ndarray) -> np.ndarray:
    x = np.ascontiguousarray(x, dtype=np.float32)
    y = np.ascontiguousarray(y, dtype=np.float32)
    std = np.ascontiguousarray(std, dtype=np.float32)
    N, M = x.shape[0], y.shape[0]
    rows = N // N_CORES
    nc = _get_nc(rows, M)
    in_maps = [
        {"x": x[c * rows : (c + 1) * rows], "y": y, "std": std}
        for c in range(N_CORES)
    ]
    trace = bool(int(os.environ.get("KERNEL_TRACE", "0")))
    res = run_bass_kernel_spmd(
        nc, in_maps, core_ids=list(range(N_CORES)), trace=trace
    )
    global LAST_RESULT
    LAST_RESULT = res
    return np.concatenate(
        [res.results[c]["out"] for c in range(N_CORES)], axis=0
    ).astype(np.float32)


LAST_RESULT = None



# revision 64
# speedup vs baseline: 1.0073x; 1.0031x over previous
"""Trainium2 Bass kernel for MatchingLayerL2:
   out = log_softmax(-sqrt(||x_i - y_j||^2) / std_j, axis=1)

x: [4096, 128] f32, y: [32768, 128] f32, std: [32768] f32 -> out [4096, 32768] f32.

Strategy: shard rows of x across 8 cores (512 rows each); y/std replicated.
Per core:
  rstd2_j = 1/std_j^2
  q_ij = rstd2_j * dist2_ij = (-2 x_i) . (y_j rstd2_j) + a_i rstd2_j + (b_j rstd2_j)
       (a = ||x||^2 rowwise, b = ||y hat||^2 * std^2 rowwise)
  s_ij = sqrt(q_ij) = dist_ij * rstd_j          (fp16 in SBUF)
  out_ij = -s_ij - ln(sum_j exp(-s_ij))          (no max-shift: s in [7,47])
Main matmul in bf16 (K=128); the rank-2 correction a*r + b*r is added with a
K=5 bf16 matmul whose rows are hi/lo bf16 splits for fp32-grade accuracy.
The 5 correction rows are staged through an internal DRAM tensor because a
[5, M] SBUF tile would charge M*2 bytes across all 128 partitions.

Schedule notes (tuned against the TimelineSim cost model; every feature
device-verified -- tensor_tensor_reduce and Pool tensor_scalar crash/wedge
the NeuronCore runtime and must not be used):
 - ACT (scalar) is the bottleneck: sqrt-from-PSUM and exp-with-accum are both
   mandatory full sweeps (~0.83 ns/elem each).  Everything else is kept under
   it: output written fp16 (halves DMA + runs the final axpy at DVE 4x),
   b-hat via one fused affine_mul_reduce per 128-column, half the yT staging
   copies on otherwise-idle ACT (Copy lives in every table set), -ln(S)
   entirely on DVE (bitcast exponent seed + cubic minimax for
   ln(m)-ln2*(m-1), |err|<=1e-3) so no table load or Newton exp sits
   between a block's last exp and the next block's Sqrt table load.
 - Engine SEQs are in-order and DMA instructions hold their queue while
   waiting, so program order ~= queue order.  Block 0's sqrt/exp (plus block
   1's first s-tile) and the corr loads are interleaved with the y-prologue
   super-chunks; rcorr chunk 0 is emitted before the first y super so its rt
   copies sit at the head of the DVE queue and ct(0) isn't starved.
 - PE matmul speed is decided at SEQ *visit* time from the length of PE's
   current busy-run (>3us full, >100ns mid, else low).  A SEQ blocked on a
   long wait re-visits with PE freshly restarted -> 788ns matmuls.  The
   phase-1 PSUM pools (3 mm bufs + transpose tiles) are closed after the
   prologue and steady state gets a 4-deep mm pool: the 4-tile runway of
   pre-computed matmuls absorbs the post-exp p-state ramp at each block
   transition.
 - SBUF is the binding constraint (~207.9 KB/partition).  Phase-1 staging
   pools live in a nested stack created last and closed after the prologue;
   the freed space hosts a 6th s-tile slot (sx_p, taken by block 1's st1)
   which decouples each block's first fresh s tile from the previous
   block's DMA-throttled out axpys (the out stream takes ~5.8us/s-tile).
 - The ~24us tail (last block's 8.4MB fp16 output after its logsumexp
   barrier) is DMA-bandwidth floor; the softmax normalizer makes it
   irreducible without approximating lnS.
"""

import os
import sys

sys.path.insert(0, "/root/.axon_site/_ro/trn_rl_repo")

import numpy as np
from contextlib import ExitStack

import concourse.bass as bass
from concourse import bacc
import concourse.tile as tile
from concourse.tile import add_dep_helper
from concourse import mybir, masks
from concourse.bass_utils import run_bass_kernel_spmd

F32 = mybir.dt.float32
BF16 = mybir.dt.bfloat16
FP16 = mybir.dt.float16
AF = mybir.ActivationFunctionType
ALU = mybir.AluOpType
AX = mybir.AxisListType

N_CORES = 8
D = 128
P = 128
# scheduling knobs (tuned against TimelineSim)
YT_MODE = 1   # yt copies: 0/1 alternate (ACT on that parity), 2 all DVE, 3 all ACT
YLOAD_SPLIT = 3   # split each y super-chunk load into (v+1) pieces
MM_BUFS = 3   # psum matmul tiles (2 banks each)
YSTAGE_BUFS = 2
OSTAGE_BUFS = 5
YBAR_BUFS = 2
ROWT_BUFS = 2
B1_EXP_AT7 = True
EARLY_JG = False
RC_ON_POOL = True
STDX_ON_POOL = False
NEWTON_ITERS = 2
PRO_EXPS = True   # emit phase-1 exps inside the prologue (False = batch after)
YT_FORCE_DVE = ()  # yt copies forced to DVE (tested: parity as-is wins)
CORRT_BUFS = 3
SX_AT = (1, 1)  # which steady s tile takes the extra 6th slot
JG0_FIRST = False  # tested: reordering jg0 before bhat g1 is worse
Y_ACT_SUPERS = 0  # tested: y loads via ACT hwdge queue are worse
STEADY_WIDE = False  # tested: 2-deep 2048 runway loses to 4-deep 1024
PIN_SQRT_TABLE = True  # dummy sqrt pins the initial act table to the Sqrt set
YB_INPLACE = False  # tested: in-place yb scale couples staging, loses 3.2us
YSTAGE_IP_BUFS = 3
PENDCT_K = 99  # prologue k at which to pre-emit block 1's ct(8); 99 = after loop
EXP_KS = (7, 15, 23, 31)  # prologue supers at which b0's s-tile exps run
ES_FP8 = True  # exp scratch output is discarded; fp8 halves its SBUF charge
BHAT_ONE_STORE = False  # tested: combined bhat store holds tp longer, +3.4us
STD1_K = -1  # std chunk 1 upfront (deferred emission tested worse)
RC0_K = 0  # super at which rcorr chunk 0 is emitted (0 = before first super)
XSIDE_K = 0  # 0 = default (k==3 for EARLY_JG=False); must be <= first-mm k
CT8_ON_SP = True  # block 1's preloaded ct(8) via the idle SP queue
CT_SP_KS = (17, 25, 31)  # prologue supers whose interior ct load rides SP
BG_SP_GS = (14, 15)  # late bhat groups' stores ride SP (idle at prologue end)
TP_BUFS = 2   # psum transpose tiles (1 bank each); 2*MM_BUFS + TP_BUFS <= 8
MM_BUFS2 = 4  # steady-state psum matmul tiles (phase-1 psum pools are closed
              # first, freeing their banks); 2*MM_BUFS2 <= 8
NEGC_DVE = True  # -ln(S) fully on DVE (bit trick + cubic); keeps the Exp->Sqrt
                 # table load off the block-transition critical path
ST_ON_DVE = False  # DVE is not an hwdge engine on TRN2; stores stay on Pool
# c(m) = ln(m) - ln2*(m-1) on [1,2): minimax cubic, |err| <= 9.3e-4
LNC_A = (-0.78590174, 1.39373203, -0.71359, 0.10668473)


def build_nc(rows, M):
    """Build the Bass module for one core: x shard [rows, D], y [M, D], std [M]."""
    NB = rows // P          # row blocks of 128
    NSUP = M // 1024        # y super-chunks (1024 rows each) == 1024-col groups
    NST = M // 8192         # s tiles per block (8192 cols each)
    nA = M // P             # layout-A columns: v[q, t] = v[t*128 + q]

    nc = bacc.Bacc("TRN2", target_bir_lowering=False, debug=False, num_swdge_queues=4)
    x_d = nc.declare_dram_parameter("x", [rows, D], F32, isOutput=False)
    y_d = nc.declare_dram_parameter("y", [M, D], F32, isOutput=False)
    std_d = nc.declare_dram_parameter("std", [M], F32, isOutput=False)
    out_d = nc.declare_dram_parameter("out", [rows, M], FP16, isOutput=True)
    corr_d = nc.dram_tensor("corr", [5, M], BF16, kind="Internal")

    act_prev = [None]

    def act(*a, **k):
        inst = nc.scalar.activation(*a, **k)
        if act_prev[0] is not None:
            add_dep_helper(inst.ins, act_prev[0].ins, sync=False, reason="act order")
        act_prev[0] = inst
        return inst

    with tile.TileContext(nc) as tc, ExitStack() as ctx:
        pool = lambda name, bufs, space="SBUF": ctx.enter_context(
            tc.tile_pool(name=name, bufs=bufs, space=space)
        )

        # Persistent pools first; phase-1 staging pools go in their own stack
        # (created last = top of the SBUF stack) so closing them after the y
        # prologue frees real address space for the extra steady-state s slot.
        const_p = pool("const", 1)
        yT_p = pool("yT", NSUP)         # 32 x [128, 1024] bf16
        lhs_p = pool("lhs", 1)
        lhsc_p = pool("lhsc", NB)
        corrt_p = pool("corrt", CORRT_BUFS)
        s_p = pool("s", NST + 1)        # 5 x [128, 8192] fp16
        part_p = pool("part", 2)
        scal_p = pool("scal", 6)
        escr_p = pool("escr", 1)
        ostage_p = pool("ostage", OSTAGE_BUFS)  # [128, 1024] fp16

        stage1 = ctx.enter_context(ExitStack())
        spool = lambda name, bufs: stage1.enter_context(
            tc.tile_pool(name=name, bufs=bufs, space="SBUF")
        )
        ystage_p = spool("ystage", YSTAGE_IP_BUFS if YB_INPLACE else YSTAGE_BUFS)
        if not YB_INPLACE:
            ybar_p = spool("ybar", YBAR_BUFS)
        sq2_p = spool("sq2", 2)         # TTR product scratch [128, 128] bf16
        colsA_p = spool("colsA", 1)     # stdA, rstdA, rA, std2A  (f32 [128, nA])
        bcols_p = spool("bcols", 1)     # b2A f32 [128, nA]
        bg_p = spool("bg", 2)           # per-group bhat tiles [128, 16]
        rowT_p = spool("rowT", ROWT_BUFS)       # transposed row chunks [*, 128] bf16
        xa_p = spool("xa", 1)
        acol_p = spool("acol", 1)

        # phase-1 PSUM pools live in their own stack: closed after the y
        # prologue so the steady-state pool can take all 8 banks (deeper PE
        # runway over the block transition hides the p-state ramp).
        psum1 = ctx.enter_context(ExitStack())
        mm_ps = psum1.enter_context(
            tc.tile_pool(name="mmps", bufs=MM_BUFS, space="PSUM")
        )  # [128,1024] f32 = 2 banks each
        tp_ps = psum1.enter_context(
            tc.tile_pool(name="tpps", bufs=TP_BUFS, space="PSUM")
        )  # [128,1024] bf16 = 1 bank each
        MM = {"ps": mm_ps}

        # ---------------- constants ----------------
        ident = const_p.tile([P, P], BF16)
        masks.make_identity(nc, ident[:])
        identf = const_p.tile([P, P], F32)
        masks.make_identity(nc, identf[:])
        if PIN_SQRT_TABLE:
            # the first real ACT op is a Copy (in every table set); without a
            # hint the initial table load picks a non-Sqrt set and a second
            # 1283ns load lands on the startup critical path before the
            # first sqrt.  A 1-element dummy Sqrt pins the right set.
            tpin = scal_p.tile([P, 1], F32, tag="h")
            act(tpin[:], identf[:, 0:1], AF.Sqrt)

        # ---------------- std-derived quantities (layout A) ----------------
        # Minimal chain to rA first: the y prologue (ybar scaling) needs it.
        # stdA[q, t] = std[128 t + q]: load natural [t, q] tiles, PE-transpose.
        stdA = colsA_p.tile([P, nA], F32)
        std2A = colsA_p.tile([P, nA], F32)
        rstdA = colsA_p.tile([P, nA], F32)
        rA = colsA_p.tile([P, nA], F32)
        stn_tiles = []

        def emit_std_chunk(c):
            # per-chunk std->rA chain: chunk 1 is deferred past the first y
            # supers (its rA cols feed supers >= 16; its stn feeds rcorr
            # chunk 1 at k==10) so it doesn't serialize ahead of y0/y1 on SP
            h = min(P, nA - c * P)
            stn = rowT_p.tile([P, P], F32, tag="stn")
            (nc.gpsimd if STDX_ON_POOL else nc.sync).dma_start(
                out=stn[0:h, :],
                in_=std_d[P * P * c : P * (P * c + h)].rearrange(
                    "(t q) -> t q", q=P
                ),
            )
            stn_tiles.append(stn)
            tpf = tp_ps.tile([P, P], F32, tag="tp")
            nc.tensor.transpose(tpf[:, 0:h], stn[0:h, :], identf[:])
            csl = slice(c * P, c * P + h)
            nc.vector.tensor_copy(stdA[:, csl], tpf[:, 0:h])
            nc.vector.tensor_tensor(
                std2A[:, csl], stdA[:, csl], stdA[:, csl], op=ALU.mult
            )
            nc.vector.reciprocal(rstdA[:, csl], stdA[:, csl])
            nc.vector.tensor_tensor(
                rA[:, csl], rstdA[:, csl], rstdA[:, csl], op=ALU.mult
            )

        emit_std_chunk(0)
        if STD1_K < 0:
            emit_std_chunk(1)
        # corr rows 0,1 = r_hi (pairs with a_hi, a_lo), row 2 = r_lo (pairs
        # a_hi).  Computed DIRECTLY on the natural-layout stn tiles (their
        # [t, q] order IS corr_d's row-major j order), so the stores depend
        # only on the std load -- not on the stdA-transpose/rA chain that
        # backs up the DVE queue at startup.
        def emit_rcorr_chunk(c):
            w = min(P, nA - c * P)
            stn = stn_tiles[c]
            rn = rowT_p.tile([P, P], F32, tag="rn")
            nc.vector.reciprocal(rn[0:w, :], stn[0:w, :])
            rq = rowT_p.tile([P, P], F32, tag="rq")
            nc.vector.tensor_tensor(rq[0:w, :], rn[0:w, :], rn[0:w, :], op=ALU.mult)
            rhi_n = rowT_p.tile([P, P], BF16, tag="rowT")
            nc.vector.tensor_copy(rhi_n[0:w, :], rq[0:w, :])
            rlo_n = rowT_p.tile([P, P], BF16, tag="rowT")
            nc.vector.tensor_tensor(
                rlo_n[0:w, :], rq[0:w, :], rhi_n[0:w, :], op=ALU.subtract
            )
            eng = nc.gpsimd if RC_ON_POOL else nc.sync
            for row, srcn in ((0, rhi_n), (1, rhi_n), (2, rlo_n)):
                eng.dma_start(
                    out=corr_d[row, c * P * P : (c * P + w) * P].rearrange(
                        "(t q) -> t q", q=P
                    ),
                    in_=srcn[0:w, :],
                )

        # ---------------- x side (emitted at phase-1 k==2) ----------------
        # lhsT_main = (-2x)^T bf16, a = ||x||^2.  Deferred into the super-chunk
        # loop so the first y super-chunks' DVE work isn't queued behind it
        # (engine queues are in-order); it's only needed by the first matmul.
        lhsT_main = lhs_p.tile([P, rows], BF16)
        lhsT_corr = []

        def emit_x_side():
            xstage = xa_p.tile([P, NB, D], F32)
            (nc.gpsimd if STDX_ON_POOL else nc.sync).dma_start(
                out=xstage[:], in_=x_d[:, :].rearrange("(c p) d -> p c d", p=P)
            )
            a_cols = acol_p.tile([P, NB], F32)
            for c in range(NB):
                xs2 = sq2_p.tile([P, D], BF16, tag="xs2")
                nc.vector.affine_mul_reduce(
                    out=xs2[:],
                    accum_out=a_cols[:, c : c + 1],
                    in0=xstage[:, c, :],
                    in1=xstage[:, c, :],
                    scale=1.0,
                    bias=0.0,
                )
            ahi_col = acol_p.tile([P, NB], BF16)
            nc.vector.tensor_copy(ahi_col[:], a_cols[:])
            alo_col = acol_p.tile([P, NB], BF16)
            nc.vector.tensor_tensor(alo_col[:], a_cols[:], ahi_col[:], op=ALU.subtract)

            xbar = xa_p.tile([P, NB, D], BF16, tag="xbar")
            nc.vector.tensor_scalar(xbar[:], xstage[:], -2.0, None, op0=ALU.mult)
            for c in range(NB):
                tp = tp_ps.tile([P, 1024], BF16, tag="tp")
                nc.tensor.transpose(tp[:, 0:P], xbar[:, c, :], ident[:])
                nc.vector.tensor_copy(lhsT_main[:, c * P : (c + 1) * P], tp[:, 0:P])

            # lhsT_corr per block: rows [a_hi; a_lo; a_hi; 1; 1] as [5, 128] bf16
            for b in range(NB):
                asm = acol_p.tile([P, 8], BF16, tag="asm")
                nc.vector.tensor_copy(asm[:, 0:1], ahi_col[:, b : b + 1])
                nc.vector.tensor_copy(asm[:, 1:2], alo_col[:, b : b + 1])
                nc.vector.tensor_copy(asm[:, 2:3], ahi_col[:, b : b + 1])
                nc.vector.memset(asm[:, 3:5], 1.0)
                tp = tp_ps.tile([P, 1024], BF16, tag="tp")
                nc.tensor.transpose(tp[0:5, 0:P], asm[:, 0:5], ident[:])
                lc = lhsc_p.tile([5, P], BF16)
                nc.vector.tensor_copy(lc[:], tp[0:5, 0:P])
                lhsT_corr.append(lc)

        # ---------------- shared emitters ----------------
        yT = []                 # 32 x [128, 1024] bf16 (super-chunk k)
        b2A = bcols_p.tile([P, nA], F32)

        def emit_y_super(k):
            """Load+scale+transpose y rows [1024k, 1024(k+1)); fill b-hat cols.

            b-hat comes from one fused affine_mul_reduce per 128-column:
            out = (yb * std2) * yb, accum = std2 * sum(yb^2) = ||yhat||^2 std^2.
            The yT staging copies alternate ACT/DVE: ACT has idle in phase 1
            (delivery-paced) and Copy lives in every activation table set."""
            yst = ystage_p.tile([P, 8, D], F32)
            # first supers ride the otherwise-idle ACT hwdge queue so the SP
            # queue (std + x) doesn't serialize the startup's y deliveries
            yq = nc.scalar if k < Y_ACT_SUPERS else nc.sync
            if YLOAD_SPLIT:
                # split loads: the ybar chain starts on the first piece
                # while the rest are still in flight
                nsp = YLOAD_SPLIT + 1
                assert 8 % nsp == 0, "YLOAD_SPLIT+1 must divide 8"
                cw = 8 // nsp
                for hh in range(nsp):
                    yq.dma_start(
                        out=yst[:, cw * hh : cw * (hh + 1), :],
                        in_=y_d[1024 * k + 128 * cw * hh : 1024 * k + 128 * cw * (hh + 1), :
                                ].rearrange("(c p) d -> p c d", p=P),
                    )
            else:
                yq.dma_start(
                    out=yst[:],
                    in_=y_d[1024 * k : 1024 * (k + 1), :].rearrange(
                        "(c p) d -> p c d", p=P
                    ),
                )
            if YB_INPLACE:
                # bf16 scale output written over the f32 stage bytes it just
                # read (per chunk: write bytes [512c,512c+256) trail the read
                # of [512c,512c+512)) -- the ybar staging pool disappears and
                # its 4KB funds a deeper ystage
                yb = yst[:].bitcast(BF16)[:, :, 0:D]
            else:
                yb = ybar_p.tile([P, 8, D], BF16)
            for c in range(8):
                nc.vector.tensor_scalar(
                    yb[:, c, :],
                    yst[:, c, :],
                    rA[:, 8 * k + c : 8 * k + c + 1],
                    None,
                    op0=ALU.mult,
                )
            for c in range(8):
                sq2 = sq2_p.tile([P, D], BF16)
                nc.vector.affine_mul_reduce(
                    out=sq2[:],
                    accum_out=b2A[:, 8 * k + c : 8 * k + c + 1],
                    in0=yb[:, c, :],
                    in1=yb[:, c, :],
                    scale=std2A[:, 8 * k + c : 8 * k + c + 1],
                    bias=0.0,
                )
            tp = tp_ps.tile([P, 1024], BF16, tag="tp")
            for c in range(8):
                nc.tensor.transpose(
                    tp[:, c * P : (c + 1) * P], yb[:, c, :], ident[:]
                )
            yt = yT_p.tile([P, 1024], BF16)
            # YT_MODE: 0/1 = alternate (ACT on that parity), 2 = all DVE,
            # 3 = all ACT.  ACT copies stay unchained: Copy lives in every
            # activation table set, and chaining would lock ACT progress to
            # y-super delivery.
            on_act = (YT_MODE == 3) or (YT_MODE in (0, 1) and k % 2 == YT_MODE)
            if k in YT_FORCE_DVE:
                # supers consumed right after a prologue exp: an ACT-queued
                # copy would sit behind the 7us exp and stall the next mms
                on_act = False
            if on_act:
                nc.scalar.copy(yt[:], tp[:])
            else:
                nc.vector.tensor_copy(yt[:], tp[:])
            yT.append(yt)

        def emit_bhat_group(g):
            """b-hat hi/lo rows for layout-A cols [16g, 16(g+1)) -> corr_d."""
            csl = slice(16 * g, 16 * (g + 1))
            bhi = bg_p.tile([P, 16], BF16, tag="bhi")
            nc.vector.tensor_copy(bhi[:], b2A[:, csl])
            blo = bg_p.tile([P, 16], BF16, tag="blo")
            nc.vector.tensor_tensor(blo[:], b2A[:, csl], bhi[:], op=ALU.subtract)
            if BHAT_ONE_STORE:
                # both rows stacked in one [32,128] tile -> a single DMA
                # (rows 3 and 4 of corr_d via a 2D pattern): halves the Pool
                # gen count feeding every ct load
                tp2 = tp_ps.tile([P, 1024], BF16, tag="tp")
                nc.tensor.transpose(tp2[0:16, 0:P], bhi[:], ident[:])
                # PE transpose writes need base partition 0/32/64
                nc.tensor.transpose(tp2[32:48, 0:P], blo[:], ident[:])
                rt = rowT_p.tile([P, P], BF16, tag="rowT")
                nc.vector.tensor_copy(rt[0:16, :], tp2[0:16, 0:P])
                nc.vector.tensor_copy(rt[16:32, :], tp2[32:48, 0:P])
                nc.gpsimd.dma_start(
                    out=corr_d[3:5, 2048 * g : 2048 * (g + 1)].rearrange(
                        "r (t q) -> r t q", q=P
                    ),
                    in_=rt[0:32, :].rearrange("(r t) q -> r t q", r=2),
                )
            else:
                for row, src in ((3, bhi), (4, blo)):
                    tp2 = tp_ps.tile([P, 1024], BF16, tag="tp")
                    nc.tensor.transpose(tp2[0:16, 0:P], src[:], ident[:])
                    rt = rowT_p.tile([P, P], BF16, tag="rowT")
                    nc.vector.tensor_copy(rt[0:16, :], tp2[0:16, 0:P])
                    (
                        nc.sync
                        if g in BG_SP_GS
                        else (nc.vector if ST_ON_DVE else nc.gpsimd)
                    ).dma_start(
                        out=corr_d[row, 2048 * g : 2048 * (g + 1)].rearrange(
                            "(t q) -> t q", q=P
                        ),
                        in_=rt[0:16, :],
                    )

        def emit_ct_load(jg, eng=None):
            """corr rows for cols [1024 jg, 1024 (jg+2)); jg even."""
            ct = corrt_p.tile([5, 2048], BF16)
            (eng or nc.gpsimd).dma_start(
                out=ct[:], in_=corr_d[:, 1024 * jg : 1024 * (jg + 2)]
            )
            return ct

        def emit_mm_pair(b, jg0, ct):
            """Main+corr matmuls for col groups jg0, jg0+1 of block b.
            The ISA caps a matmul's moving free dim at 512, so each 1024-col
            group is two matmuls; all four mains run back-to-back before the
            four corrs so the stationary tensor only switches once per pair."""
            mms = []
            for i in range(2):
                mm = MM["ps"].tile([P, 1024], F32, tag="mm")
                for q in range(2):
                    nc.tensor.matmul(
                        mm[:, 512 * q : 512 * (q + 1)],
                        lhsT_main[:, b * P : (b + 1) * P],
                        yT[jg0 + i][:, 512 * q : 512 * (q + 1)],
                        start=True,
                        stop=False,
                    )
                mms.append(mm)
            for i in range(2):
                for q in range(2):
                    nc.tensor.matmul(
                        mms[i][:, 512 * q : 512 * (q + 1)],
                        lhsT_corr[b][:],
                        ct[:, 1024 * i + 512 * q : 1024 * i + 512 * (q + 1)],
                        start=False,
                        stop=True,
                    )
            return mms

        def emit_mm_quad(b, jg0, ct):
            """Steady-state variant: one [128, 2048] psum tile covering col
            groups jg0, jg0+1 (exactly one ct tile), consumed by ONE
            2048-wide sqrt -- halves the per-sqrt ACT access-latency
            charge."""
            mm = MM["ps"].tile([P, 2048], F32, tag="mm")
            for i in range(4):
                nc.tensor.matmul(
                    mm[:, 512 * i : 512 * (i + 1)],
                    lhsT_main[:, b * P : (b + 1) * P],
                    yT[jg0 + i // 2][:, 512 * (i % 2) : 512 * (i % 2 + 1)],
                    start=True,
                    stop=False,
                )
            for i in range(4):
                nc.tensor.matmul(
                    mm[:, 512 * i : 512 * (i + 1)],
                    lhsT_corr[b][:],
                    ct[:, 512 * i : 512 * (i + 1)],
                    start=False,
                    stop=True,
                )
            return mm

        def emit_sqrt(s_t, h, mm):
            act(s_t[:, 1024 * h : 1024 * (h + 1)], mm[:], AF.Sqrt)

        def emit_sqrt2(s_t, h2, mm):
            act(s_t[:, 2048 * h2 : 2048 * (h2 + 1)], mm[:], AF.Sqrt)

        def emit_exp(s_t, partials, st):
            es = escr_p.tile([P, 8192], mybir.dt.float8e4 if ES_FP8 else BF16)
            act(
                es[:],
                s_t[:],
                AF.Exp,
                scale=-1.0,
                accum_out=partials[:, st : st + 1],
            )

        LN2 = 0.6931471805599453

        def emit_lnS(partials):
            # negc = -ln(S), entirely on DVE so no Ln/Exp table dependency:
            # S = m 2^E; bits/2^23 = (E+127) + (m-1);
            # ln S = ln2*(bits/2^23 - 127) + c(m), c = ln m - ln2 (m-1)
            # (cubic minimax for c, |err| <= 9.3e-4 -> ~5e-5 rel on output).
            S = scal_p.tile([P, 1], F32)
            nc.vector.tensor_reduce(S[:], partials[:], axis=AX.X, op=ALU.add)
            if not NEGC_DVE:
                bits = scal_p.tile([P, 1], F32)
                nc.vector.tensor_copy(bits[:], S[:].bitcast(mybir.dt.int32))
                y = scal_p.tile([P, 1], F32, tag="y")
                nc.vector.tensor_scalar(
                    y[:], bits[:], LN2 / (1 << 23), -(127.0 - 0.043) * LN2,
                    op0=ALU.mult, op1=ALU.add,
                )
                for _ in range(NEWTON_ITERS):
                    e = scal_p.tile([P, 1], F32, tag="nwe")
                    act(e[:], y[:], AF.Exp, scale=-1.0)
                    t = scal_p.tile([P, 1], F32, tag="nwt")
                    nc.vector.tensor_tensor(t[:], e[:], S[:], op=ALU.mult)
                    y2 = scal_p.tile([P, 1], F32, tag="y")
                    nc.vector.tensor_tensor(y2[:], y[:], t[:], op=ALU.add)
                    y = scal_p.tile([P, 1], F32, tag="y")
                    nc.vector.tensor_scalar(y[:], y2[:], 1.0, None, op0=ALU.subtract)
                negc = scal_p.tile([P, 1], F32)
                nc.vector.tensor_scalar(negc[:], y[:], -1.0, None, op0=ALU.mult)
                return negc
            a0, a1, a2, a3 = LNC_A
            bits = scal_p.tile([P, 1], F32)
            nc.vector.tensor_copy(bits[:], S[:].bitcast(mybir.dt.int32))
            nb = scal_p.tile([P, 1], F32, tag="nb")
            nc.vector.tensor_scalar(
                nb[:], bits[:], -LN2 / (1 << 23), 127.0 * LN2,
                op0=ALU.mult, op1=ALU.add,
            )
            mb = scal_p.tile([P, 1], mybir.dt.int32, tag="mb")
            nc.vector.tensor_scalar(
                mb[:], S[:].bitcast(mybir.dt.int32), 0x007FFFFF, 0x3F800000,
                op0=ALU.bitwise_and, op1=ALU.bitwise_or,
            )
            m = mb[:].bitcast(F32)
            t = scal_p.tile([P, 1], F32, tag="h")
            nc.vector.tensor_scalar(t[:], m, -a3, -a2, op0=ALU.mult, op1=ALU.add)
            t2 = scal_p.tile([P, 1], F32, tag="h")
            nc.vector.tensor_tensor(t2[:], t[:], m, op=ALU.mult)
            t3 = scal_p.tile([P, 1], F32, tag="h")
            nc.vector.tensor_scalar(t3[:], t2[:], -a1, None, op0=ALU.add)
            t4 = scal_p.tile([P, 1], F32, tag="h")
            nc.vector.tensor_tensor(t4[:], t3[:], m, op=ALU.mult)
            t5 = scal_p.tile([P, 1], F32, tag="h")
            nc.vector.tensor_scalar(t5[:], t4[:], -a0, None, op0=ALU.add)
            negc = scal_p.tile([P, 1], F32)
            nc.vector.tensor_tensor(negc[:], nb[:], t5[:], op=ALU.add)
            return negc

        def emit_out(b, s_tiles, negc):
            for st in range(NST):
                for h in range(8):
                    og = ostage_p.tile([P, 1024], FP16)
                    nc.vector.tensor_scalar(
                        og[:],
                        s_tiles[st][:, 1024 * h : 1024 * (h + 1)],
                        -1.0,
                        negc[:],
                        op0=ALU.mult,
                        op1=ALU.add,
                    )
                    j0 = 8192 * st + 1024 * h
                    nc.sync.dma_start(
                        out=out_d[b * P : (b + 1) * P, j0 : j0 + 1024],
                        in_=og[:],
                    )

        # ---------------- phase 1: y prologue + block 0 (and b1 st0) ----------------
        # The first sqrts wait for the corr roundtrip (bhat group 0 store ->
        # ct load), so jg 0..3 are emitted as one batch at k==3; block 1's
        # first s-tile rides the same ct/yT deliveries (k < 8) to fill ACT.
        s0_tiles = []
        pending = [None]
        partials0 = part_p.tile([P, NST], F32)
        partials1 = part_p.tile([P, NST], F32)
        b1_st0 = None
        s_t = None
        for k in range(NSUP):
            if k == 0 and RC0_K == 0:
                # before the first y super: the rcorr rt copies land at the
                # head of the DVE queue (ahead of the y scales) so the Pool
                # stores' wait-at-head is short and ct(0) isn't starved
                emit_rcorr_chunk(0)
            emit_y_super(k)
            if k == STD1_K:
                emit_std_chunk(1)
            if k == 10:
                emit_rcorr_chunk(1)
            if k % 2 == 1:
                if not (JG0_FIRST and not EARLY_JG and k == 3):
                    emit_bhat_group((k - 1) // 2)
                if k == RC0_K:
                    # rcorr chunk 0 has ~8us of slack before ct(0); emitting
                    # it after the bhat-critical scales/amrs tests whether
                    # the DVE queue head is better spent on the bhat chain
                    emit_rcorr_chunk(0)
                if k == (XSIDE_K if XSIDE_K else (1 if EARLY_JG else 3)):
                    # after bhat group 0's DVE work so the corr roundtrip
                    # (which gates the first sqrt) isn't queued behind the
                    # x-side DVE chain
                    emit_x_side()
                if EARLY_JG and k == 1:
                    jgr = range(0, 2)
                elif k < 3:
                    continue
                elif k == 3:
                    jgr = range(0, 4) if not EARLY_JG else range(2, 4)
                else:
                    jgr = range(k - 1, k + 1)
                for jg in jgr:
                    if jg % 2 == 1:
                        continue
                    if JG0_FIRST and not EARLY_JG and k == 3 and jg == 2:
                        # ct(0) only needs bhat group 0: the jg 0-1 batch is
                        # emitted before group 1's stores so ct(0) isn't
                        # queued on Pool behind stores waiting on supers 2-3
                        emit_bhat_group(1)
                    ct = emit_ct_load(
                        jg, eng=nc.sync if k in CT_SP_KS else None
                    )
                    if jg % 8 == 0:
                        s_t = s_p.tile([P, 8192], FP16, tag="s_t")
                        s0_tiles.append(s_t)
                        if jg == 0:
                            # allocated after s0's first tile: pool rotation
                            # then lands block 1's later tiles on slots that
                            # free early
                            b1_st0 = s_p.tile([P, 8192], FP16, tag="s_t")
                    mms = emit_mm_pair(0, jg, ct)
                    emit_sqrt(s_t, jg % 8, mms[0])
                    emit_sqrt(s_t, jg % 8 + 1, mms[1])
                    if k < 8:
                        # block 1's first s-tile rides the same deliveries
                        mms1 = emit_mm_pair(1, jg, ct)
                        emit_sqrt(b1_st0, jg % 8, mms1[0])
                        emit_sqrt(b1_st0, jg % 8 + 1, mms1[1])
                if k == PENDCT_K:
                    # block 1's first steady ct: emitted mid-prologue so it
                    # sits AHEAD of the late cts (26..30) on the Pool queue;
                    # emitted at the back it transfers only after the k=31
                    # exp and block 1's corr matmuls restart cold
                    pending[0] = emit_ct_load(8)
                if PRO_EXPS == 1 and k in EXP_KS:
                    st_i = EXP_KS.index(k)
                    emit_exp(s0_tiles[st_i], partials0, st_i)
                    if st_i == 0:
                        emit_exp(b1_st0, partials1, 0)
                elif PRO_EXPS == 2 and k in (15, 31):
                    # pairs: one Exp-table residency per two s tiles
                    base = (k - 15) // 16 * 2
                    emit_exp(s0_tiles[base], partials0, base)
                    emit_exp(s0_tiles[base + 1], partials0, base + 1)
                    if k == 31:
                        emit_exp(b1_st0, partials1, 0)
        # block 1 resumes at jg 8; pre-load its corr tile while ACT runs the
        # phase-1 exp tail so the first steady-state sqrt isn't DMA-gated
        # ct(8) rides the now-idle SP queue: on Pool it sits behind the
        # head-of-line slot-waits of ct(26..30) and transfers only after the
        # k=31 exp, restarting block 1's corr matmuls cold
        pending_ct = (
            pending[0]
            if pending[0] is not None
            else emit_ct_load(8, eng=nc.sync if CT8_ON_SP else None)
        )
        # phase-1 PSUM (3 mm bufs + transpose tiles) closes here; steady
        # state gets a 4-deep mm pool so PE can pre-run a whole extra jg
        # pair across each block transition.  The SBUF staging pools close
        # too, freeing room for a 6th s-tile slot (sx_p) that decouples each
        # block's first fresh s tile from the previous block's out axpys.
        psum1.close()
        stage1.close()
        MM["ps"] = ctx.enter_context(
            tc.tile_pool(
                name="mmps2",
                bufs=2 if STEADY_WIDE else MM_BUFS2,
                space="PSUM",
            )
        )
        sx_p = ctx.enter_context(tc.tile_pool(name="sx", bufs=1, space="SBUF"))
        if not PRO_EXPS:
            # batched phase-1 exps: one Exp table residency instead of three
            # Sqrt<->Exp round trips inside the prologue
            for st in range(NST):
                emit_exp(s0_tiles[st], partials0, st)
            emit_exp(b1_st0, partials1, 0)
        negc0 = emit_lnS(partials0)
        emit_out(0, s0_tiles, negc0)

        # ---------------- blocks 1..NB-1 ----------------
        for b in range(1, NB):
            if b == 1:
                partials = partials1
                s_tiles = [b1_st0]
                st_range = range(1, NST)
            else:
                partials = part_p.tile([P, NST], F32)
                s_tiles = []
                st_range = range(NST)
            first_jg = 8 * st_range.start
            for st in st_range:
                sp = sx_p if (b, st) == SX_AT else s_p
                s_t = sp.tile([P, 8192], FP16, tag="s_t", name="s_t")
                for h2 in range(4):
                    jg = 8 * st + 2 * h2
                    ct = pending_ct if jg == first_jg else emit_ct_load(jg)
                    if STEADY_WIDE:
                        mm = emit_mm_quad(b, jg, ct)
                        emit_sqrt2(s_t, h2, mm)
                    else:
                        mms = emit_mm_pair(b, jg, ct)
                        emit_sqrt(s_t, 2 * h2, mms[0])
                        emit_sqrt(s_t, 2 * h2 + 1, mms[1])
                s_tiles.append(s_t)
            if b < NB - 1:
                # pre-load the next block's first corr tile during this
                # block's exp phase (SP queue: past out(b-1)'s gens, clear of
                # the Pool queue's tail-of-block slot-waits)
                pending_ct = emit_ct_load(0, eng=nc.sync if CT8_ON_SP else None)
            for st in st_range:
                emit_exp(s_tiles[st], partials, st)
            negc = emit_lnS(partials)
            emit_out(b, s_tiles, negc)

    nc.finalize()
    return nc


_NC_CACHE = {}


def _get_nc(rows, M):
    key = (rows, M)
    if key not in _NC_CACHE:
        _NC_CACHE[key] = build_nc(rows, M)
    return _NC_CACHE[key]


def kernel(x: np.ndarray, y: np.ndarray, std: np.ndarray) -> np.ndarray:
    x = np.ascontiguousarray(x, dtype=np.float32)
    y = np.ascontiguousarray(y, dtype=np.float32)
    std = np.ascontiguousarray(std, dtype=np.float32)
    N, M = x.shape[0], y.shape[0]
    rows = N // N_CORES
    nc = _get_nc(rows, M)
    in_maps = [
        {"x": x[c * rows : (c + 1) * rows], "y": y, "std": std}
        for c in range(N_CORES)
    ]
    trace = bool(int(os.environ.get("KERNEL_TRACE", "0")))
    res = run_bass_kernel_spmd(
        nc, in_maps, core_ids=list(range(N_CORES)), trace=trace
    )
    global LAST_RESULT
    LAST_RESULT = res
    return np.concatenate(
        [res.results[c]["out"] for c in range(N_CORES)], axis=0
    ).astype(np.float32)


LAST_RESULT = None

